# revision 12
# baseline (speedup 1.0000x reference)
"""Trainium2 Bass kernel for nn_Decoder (ragged sequence decoder).

Reference math:
  n      = clip(round(size_pred(z)), 0, 128)            [B]  (tiny scalar head)
  keys   = mish(LN(kn_W1 + kn_b1)) @ kn_W2 + kn_b2      [128, 512]
  x      = mish((z[:,None,:] * keys[None]) @ de_W1 + de_b1) @ de_W2 + de_b2
  x     *= (arange(128) < n[:,None])[..., None]         zero padded slots

Strategy: pure data parallel over batch (8 cores x 256 rows). The size head
(0.3% of FLOPs) runs on host in fp32 to build the ragged schedule: only
positions k < max(n) are computed on device; the rest of the output stays
zero via the pre-zeroed output buffers. keys are computed redundantly on
each device. The big decoder runs with fp16 matmul inputs and fp32 PSUM
accumulation. Per position k: x[:,k,:] = mish((z * keys[k]) @ W1) @ W2,
i.e. scale zT columns-of-keys into the moving matmul operand.

mish(x) = x*tanh(softplus(x)) has no HW activation table; it is computed
exactly via g = sigmoid(-x):  tanh(softplus(x)) = (1-g^2)/(1+g^2), so
  mish(x) = (r - 1) * x   with  r = 2/(1 + g^2)
using one ScalarE pass (Sigmoid), GpSimd passes (g^2, 0.5+0.5*g^2), and DVE
passes (reciprocal, fused (r-1)*x via scalar_tensor_tensor).
"""

import numpy as np
from contextlib import ExitStack

import concourse.bass as bass
import concourse.tile as tile
from concourse import bacc, mybir
from concourse.bass_utils import run_bass_kernel_spmd
from concourse.masks import make_identity

AF = mybir.ActivationFunctionType
ALU = mybir.AluOpType
DT = mybir.dt

B, DIM, HID, MAXN = 2048, 256, 512, 128
MID_S = (HID + 1) // 2      # 256
MID_K = (MAXN + HID) // 2   # 320
MID_D = (HID + DIM) // 2    # 384
NCORES = 8
BS = B // NCORES            # 256 rows per core
HC = HID // 128             # 4 h-chunks
DC = MID_D // 128           # 3 d-chunks
LN_EPS = 1e-5

LAST_RUN = None  # BassKernelResults of the last device launch (for profiling)


# ----------------------------------------------------------------- host math
def _np_mish(x):
    return (x * np.tanh(np.log1p(np.exp(x)))).astype(np.float32)


def _np_ln(x, g, b):
    m = x.mean(-1, keepdims=True, dtype=np.float32)
    v = x.var(-1, keepdims=True, dtype=np.float32)
    return ((x - m) / np.sqrt(v + LN_EPS) * g + b).astype(np.float32)


def _host_size_pred(z, sp_W1, sp_b1, sp_g, sp_be, sp_W2, sp_b2):
    h = _np_mish(_np_ln((z @ sp_W1 + sp_b1).astype(np.float32), sp_g, sp_be))
    nl = (h @ sp_W2 + sp_b2).astype(np.float32)
    return np.clip(np.round(nl[:, 0]), 0, MAXN).astype(np.int32)


# ------------------------------------------------------------- device kernel
def _act_recip(nc, out, in_, scale, bias):
    """ScalarE table reciprocal: out = 1/(in*scale + bias).

    bass.activation() refuses Reciprocal wholesale (generic accuracy
    concerns); our input range is exactly [1, 2] where the 1016-bucket
    table is accurate to ~1e-5 rel (hardware-probed), so emit directly.
    """
    eng = nc.scalar
    ins = [eng.lower_ap(in_)]
    for arg in (bias, scale, 0.0):
        ins.append(mybir.ImmediateValue(dtype=mybir.dt.float32, value=arg))
    return eng.add_instruction(mybir.InstActivation(
        name=nc.get_next_instruction_name(), func=AF.Reciprocal,
        ins=ins, outs=[eng.lower_ap(out)]))


def _emit_mish(nc, pools, frags, parts, width):
    """mish over PSUM fp32 fragments [(x_psum_ap, out16_ap), ...].

    mish(x) = (r - 1) * x,  r = 1/(0.5*sigmoid(-x)^2 + 0.5)
    """
    g = pools["mg"].tile([parts, width], DT.float32, tag="mg")
    q = pools["mq"].tile([parts, width], DT.float32, tag="mq")
    r = pools["mr"].tile([parts, width], DT.float32, tag="mr")
    off = 0
    for x_psum, out16, w in frags:
        gs, qs, rs = g[:, off:off + w], q[:, off:off + w], r[:, off:off + w]
        nc.scalar.activation(gs, x_psum, AF.Sigmoid, scale=-1.0)
        nc.gpsimd.tensor_tensor(qs, gs, gs, ALU.mult)
        _act_recip(nc, rs, qs, 0.5, 0.5)
        nc.vector.scalar_tensor_tensor(out16, rs, 1.0, x_psum, ALU.subtract,
                                       ALU.mult)
        off += w


def build_kernel(K, de_b1_nonzero, de_b2_nonzero, kn_b2_nonzero, kn_skip,
                 repeat=1):
    """One SPMD bass program computing x[:, :K, :] for a 256-row shard.

    repeat > 1 wraps the main loop in a hardware For-loop re-running the
    same (idempotent) computation; used only for wall-clock benchmarking.
    """
    nc = bacc.Bacc("TRN2", target_bir_lowering=False, debug=False,
                   num_devices=NCORES)

    z_d = nc.dram_tensor("z", [BS, HID], DT.float32, kind="ExternalInput").ap()
    maskf_d = nc.dram_tensor("maskf", [BS, MAXN], DT.float32,
                             kind="ExternalInput").ap()
    knW1_d = nc.dram_tensor("kn_W1", [MAXN, MID_K], DT.float32,
                            kind="ExternalInput").ap()
    knW2_d = nc.dram_tensor("kn_W2", [MID_K, HID], DT.float32,
                            kind="ExternalInput").ap()
    knb2_d = nc.dram_tensor("kn_b2", [HID], DT.float32, kind="ExternalInput").ap()
    deW1_d = nc.dram_tensor("de_W1", [HID, MID_D], DT.float32,
                            kind="ExternalInput").ap()
    deb1_d = nc.dram_tensor("de_b1", [MID_D], DT.float32, kind="ExternalInput").ap()
    deW2_d = nc.dram_tensor("de_W2", [MID_D, DIM], DT.float32,
                            kind="ExternalInput").ap()
    deb2_d = nc.dram_tensor("de_b2", [DIM], DT.float32, kind="ExternalInput").ap()
    x_d = nc.dram_tensor("x", [BS, MAXN, DIM], DT.float32,
                         kind="ExternalOutput").ap()

    with tile.TileContext(nc) as tc, ExitStack() as ctx:
        wpool = ctx.enter_context(tc.tile_pool(name="weights", bufs=1))
        mish_pools = {nm: ctx.enter_context(tc.tile_pool(name=nm, bufs=2))
                      for nm in ("mg", "mq", "mr")}

        ident = wpool.tile([128, 128], DT.float32, tag="ident")
        make_identity(nc, ident[:])

        with tc.tile_pool(name="psetup", bufs=2, space="PSUM") as psetup, \
             tc.tile_pool(name="setup_sb", bufs=2) as ssb:
            # --- persistent weights (fp16)
            W1h = []   # de_W1, 4 x [128h, 384d]
            for hc in range(HC):
                t32 = ssb.tile([128, MID_D], DT.float32, tag="w1stage")
                nc.sync.dma_start(t32[:], deW1_d[hc * 128:(hc + 1) * 128, :])
                t16 = wpool.tile([128, MID_D], DT.float16, tag=f"w1h{hc}")
                nc.vector.tensor_copy(t16[:], t32[:])
                W1h.append(t16)
            W2h = []   # de_W2, 3 x [128d, 256]
            for dc in range(DC):
                t32 = ssb.tile([128, DIM], DT.float32, tag="w2stage")
                nc.sync.dma_start(t32[:], deW2_d[dc * 128:(dc + 1) * 128, :])
                t16 = wpool.tile([128, DIM], DT.float16, tag=f"w2h{dc}")
                nc.vector.tensor_copy(t16[:], t32[:])
                W2h.append(t16)

            deb1c = []
            if de_b1_nonzero:
                for dc in range(DC):
                    t = wpool.tile([128, 1], DT.float32, tag=f"deb1{dc}")
                    nc.sync.dma_start(t[:], deb1_d[dc * 128:(dc + 1) * 128][:, None])
                    deb1c.append(t)
            deb2row = ones_row = None
            if de_b2_nonzero:
                t32 = ssb.tile([1, DIM], DT.float32, tag="deb2st")
                nc.sync.dma_start(t32[:], deb2_d[None, :])
                deb2row = wpool.tile([1, DIM], DT.float16, tag="deb2h")
                nc.vector.tensor_copy(deb2row[:], t32[:])
                ones_row = wpool.tile([1, 128], DT.float16, tag="ones")
                nc.vector.memset(ones_row[:], 1.0)

            maskf = []  # 2 x [128b, 128k] fp32
            for bt in range(2):
                t = wpool.tile([128, MAXN], DT.float32, tag=f"maskf{bt}")
                nc.sync.dma_start(t[:], maskf_d[bt * 128:(bt + 1) * 128, :])
                maskf.append(t)

            # --- transpose z -> zTh fp16 [128h, 256b] x4
            zTh = [wpool.tile([128, BS], DT.float16, tag=f"zth{hc}", name=f"zth{hc}")
                   for hc in range(HC)]
            for bt in range(2):
                zrow = ssb.tile([128, HID], DT.float32, tag="zrow")
                nc.sync.dma_start(zrow[:], z_d[bt * 128:(bt + 1) * 128, :])
                for hc in range(HC):
                    pt = psetup.tile([128, 128], DT.float32, tag="ptr")
                    nc.tensor.transpose(pt[:], zrow[:, hc * 128:(hc + 1) * 128],
                                        ident[:])
                    nc.scalar.activation(zTh[hc][:, bt * 128:(bt + 1) * 128],
                                         pt[:], AF.Identity)

            # --- keys: km = mish(LN(kn_W1eff))   (kn_b1 folded on host)
            kw1 = ssb.tile([MAXN, MID_K], DT.float32, tag="kw1")
            nc.sync.dma_start(kw1[:], knW1_d)
            if kn_skip:
                km = kw1
            else:
                mean = ssb.tile([128, 1], DT.float32, tag="mean")
                nc.vector.tensor_reduce(mean[:], kw1[:], mybir.AxisListType.X,
                                        ALU.add)
                sq = ssb.tile([MAXN, MID_K], DT.float32, tag="sq")
                ssq = ssb.tile([128, 1], DT.float32, tag="ssq")
                nc.scalar.activation(sq[:], kw1[:], AF.Square, accum_out=ssq[:])
                nc.scalar.mul(mean[:], mean[:], 1.0 / MID_K)
                m2 = ssb.tile([128, 1], DT.float32, tag="m2")
                nc.scalar.activation(m2[:], mean[:], AF.Square)
                var = ssb.tile([128, 1], DT.float32, tag="var")
                nc.vector.scalar_tensor_tensor(var[:], ssq[:], 1.0 / MID_K,
                                               m2[:], ALU.mult, ALU.subtract)
                epsc = ssb.tile([128, 1], DT.float32, tag="epsc")
                nc.gpsimd.memset(epsc[:], LN_EPS)
                srt = ssb.tile([128, 1], DT.float32, tag="srt")
                nc.scalar.activation(srt[:], var[:], AF.Sqrt, bias=epsc[:, 0:1])
                rstd = ssb.tile([128, 1], DT.float32, tag="rstd")
                nc.vector.reciprocal(rstd[:], srt[:])
                nrm = ssb.tile([MAXN, MID_K], DT.float32, tag="nrm")
                nc.vector.tensor_scalar(nrm[:], kw1[:], mean[:, 0:1],
                                        rstd[:, 0:1], ALU.subtract, ALU.mult)
                # mish(nrm) in fp32 (same chain, fp32 output)
                g = mish_pools["mg"].tile([MAXN, MID_K], DT.float32, tag="mg")
                nc.scalar.activation(g[:], nrm[:], AF.Sigmoid, scale=-1.0)
                q = mish_pools["mq"].tile([MAXN, MID_K], DT.float32, tag="mq")
                nc.gpsimd.tensor_tensor(q[:], g[:], g[:], ALU.mult)
                r = mish_pools["mr"].tile([MAXN, MID_K], DT.float32, tag="mr")
                _act_recip(nc, r[:], q[:], 0.5, 0.5)
                km = ssb.tile([MAXN, MID_K], DT.float32, tag="km")
                nc.vector.scalar_tensor_tensor(km[:], r[:], 1.0, nrm[:],
                                               ALU.subtract, ALU.mult)

            # kmT chunks: 320m -> [128,128,64] partitions x [128k]
            mc_sizes = [128, 128, MID_K - 256]
            kmT = []
            for mc, msz in enumerate(mc_sizes):
                pt = psetup.tile([msz, 128], DT.float32, tag="pkmt")
                nc.tensor.transpose(pt[:], km[:, mc * 128:mc * 128 + msz],
                                    ident[:])
                t = ssb.tile([msz, 128], DT.float32, tag=f"kmt{mc}")
                nc.scalar.activation(t[:], pt[:], AF.Identity)
                kmT.append(t)

            knW2t = []
            for mc, msz in enumerate(mc_sizes):
                t = ssb.tile([msz, HID], DT.float32, tag=f"kw2{mc}")
                nc.sync.dma_start(t[:], knW2_d[mc * 128:mc * 128 + msz, :])
                knW2t.append(t)
            keysT = []   # 4 x [128h, 128k] fp32
            for hc in range(HC):
                pk = psetup.tile([128, 128], DT.float32, tag="pkeys")
                for mc, msz in enumerate(mc_sizes):
                    nc.tensor.matmul(pk[:],
                                     knW2t[mc][:, hc * 128:(hc + 1) * 128],
                                     kmT[mc][:], start=(mc == 0), stop=(mc == 2))
                kt = wpool.tile([128, MAXN], DT.float32, tag=f"keysT{hc}")
                if kn_b2_nonzero:
                    kb = ssb.tile([128, 1], DT.float32, tag="knb2c")
                    nc.sync.dma_start(kb[:],
                                      knb2_d[hc * 128:(hc + 1) * 128][:, None])
                    nc.scalar.activation(kt[:], pk[:], AF.Identity,
                                         bias=kb[:, 0:1])
                else:
                    nc.scalar.activation(kt[:], pk[:], AF.Identity)
                keysT.append(kt)

        # ------------------------------------------------------ main k loop
        ps1 = ctx.enter_context(tc.tile_pool(name="ps1", bufs=2, space="PSUM"))
        ps2 = ctx.enter_context(tc.tile_pool(name="ps2", bufs=2, space="PSUM"))
        zkp = ctx.enter_context(tc.tile_pool(name="zk", bufs=3))
        actp = ctx.enter_context(tc.tile_pool(name="act", bufs=2))
        outp = ctx.enter_context(tc.tile_pool(name="outsb", bufs=3))

        def _main_body():
          for k0 in range(0, K, 2):
            nk = min(2, K - k0)
            W = nk * BS
            # zk[hc] = zTh[hc] * keysT[hc][:, k]  (GpSimd, fp16, fp32 scalar)
            zk = [zkp.tile([128, W], DT.float16, tag=f"zk{hc}", name=f"zk{hc}")
                  for hc in range(HC)]
            for kk in range(nk):
                for hc in range(HC):
                    nc.gpsimd.tensor_scalar(
                        zk[hc][:, kk * BS:(kk + 1) * BS], zTh[hc][:],
                        keysT[hc][:, k0 + kk:k0 + kk + 1], None, ALU.mult)
            # mm1 into one fused 3-bank psum tile [128, 3*512]; dc chunks sit
            # at fixed 512-column (one-bank) strides so each accumulation
            # group owns a bank even when nk == 1.
            p1 = ps1.tile([128, DC * 512], DT.float32, tag="p1")
            for dc in range(DC):
                sl = p1[:, dc * 512:dc * 512 + W]
                for hc in range(HC):
                    nc.tensor.matmul(sl, W1h[hc][:, dc * 128:(dc + 1) * 128],
                                     zk[hc][:], start=(hc == 0),
                                     stop=(hc == HC - 1))
                if de_b1_nonzero:
                    nc.vector.tensor_scalar(sl, sl, deb1c[dc][:, 0:1],
                                            None, ALU.add)
            a16 = actp.tile([128, DC * W], DT.float16, tag="a16")
            if nk == 2:
                frags = [(p1[:, :DC * 512], a16[:, :DC * 512], DC * 512)]
            else:
                frags = [(p1[:, dc * 512:dc * 512 + W],
                          a16[:, dc * W:(dc + 1) * W], W) for dc in range(DC)]
            _emit_mish(nc, mish_pools, frags, 128, DC * 512)
            out_sb = [outp.tile([128, W], DT.float32, tag=f"o{bt}", name=f"o{bt}")
                      for bt in range(2)]
            for kk in range(nk):
                for bt in range(2):
                    p2 = ps2.tile([128, DIM], DT.float32, tag="p2")
                    for dc in range(DC):
                        last = (dc == DC - 1) and not de_b2_nonzero
                        nc.tensor.matmul(
                            p2[:],
                            a16[:, dc * W + kk * BS + bt * 128:
                                dc * W + kk * BS + (bt + 1) * 128],
                            W2h[dc][:], start=(dc == 0), stop=last)
                    if de_b2_nonzero:
                        nc.tensor.matmul(p2[:], ones_row[:], deb2row[:],
                                         start=False, stop=True)
                    mcol = maskf[bt][:, k0 + kk:k0 + kk + 1]
                    dst = out_sb[bt][:, kk * DIM:(kk + 1) * DIM]
                    nc.vector.tensor_scalar(dst, p2[:], mcol, None, ALU.mult)
            for bt in range(2):
                nc.sync.dma_start(
                    x_d[bt * 128:(bt + 1) * 128, k0:k0 + nk, :],
                    out_sb[bt][:, :W])

        if repeat > 1:
            with tc.For_i(0, repeat, 1):
                _main_body()
        else:
            _main_body()

    nc.compile()
    return nc


_NEFF_CACHE = {}


def kernel(**inputs):
    global LAST_RUN
    inp = {k: np.ascontiguousarray(np.asarray(v)) for k, v in inputs.items()}
    z = np.ascontiguousarray(inp["z"].astype(np.float32, copy=False))

    # --- host: size head -> n, mask, ragged bound K
    n = _host_size_pred(z, inp["sp_W1"], inp["sp_b1"], inp["sp_g"],
                        inp["sp_be"], inp["sp_W2"], inp["sp_b2"])
    mask = np.arange(MAXN)[None, :] < n[:, None]
    maskf = np.ascontiguousarray(mask.astype(np.float32))
    K = int(n.max())
    x = np.zeros((B, MAXN, DIM), np.float32)
    if K == 0:
        return x, n, mask

    # fold kn_b1 into kn_W1; if LN affine is non-trivial, fold the whole
    # key_net hidden layer on host (device then skips LN+mish).
    kn_w1eff = (inp["kn_W1"] + inp["kn_b1"]).astype(np.float32)
    kn_skip = False
    if not (np.all(inp["kn_g"] == 1.0) and np.all(inp["kn_be"] == 0.0)):
        kn_w1eff = _np_mish(_np_ln(kn_w1eff, inp["kn_g"], inp["kn_be"]))
        kn_skip = True

    de_b1_nonzero = bool(np.any(inp["de_b1"] != 0))
    de_b2_nonzero = bool(np.any(inp["de_b2"] != 0))
    kn_b2_nonzero = bool(np.any(inp["kn_b2"] != 0))

    ck = (K, de_b1_nonzero, de_b2_nonzero, kn_b2_nonzero, kn_skip)
    if ck not in _NEFF_CACHE:
        _NEFF_CACHE[ck] = build_kernel(*ck)
    nc = _NEFF_CACHE[ck]

    f32 = lambda a: np.ascontiguousarray(a.astype(np.float32, copy=False))
    shared = {
        "kn_W1": f32(kn_w1eff),
        "kn_W2": f32(inp["kn_W2"]),
        "kn_b2": f32(inp["kn_b2"]),
        "de_W1": f32(inp["de_W1"]),
        "de_b1": f32(inp["de_b1"]),
        "de_W2": f32(inp["de_W2"]),
        "de_b2": f32(inp["de_b2"]),
    }
    in_maps = [{**shared, "z": z[c * BS:(c + 1) * BS],
                "maskf": maskf[c * BS:(c + 1) * BS]} for c in range(NCORES)]

    LAST_RUN = run_bass_kernel_spmd(nc, in_maps, list(range(NCORES)))
    for c in range(NCORES):
        x[c * BS:(c + 1) * BS] = np.asarray(LAST_RUN.results[c]["x"])
    return x, n, mask


# revision 13
# speedup vs baseline: 1.2107x; 1.2107x over previous
"""Trainium2 Bass kernel for nn_Decoder (ragged sequence decoder).

Reference math:
  n      = clip(round(size_pred(z)), 0, 128)            [B]  (tiny scalar head)
  keys   = mish(LN(kn_W1 + kn_b1)) @ kn_W2 + kn_b2      [128, 512]
  x      = mish((z[:,None,:] * keys[None]) @ de_W1 + de_b1) @ de_W2 + de_b2
  x     *= (arange(128) < n[:,None])[..., None]         zero padded slots

Strategy: pure data parallel over batch (8 cores x 256 rows). The size head
(0.3% of FLOPs) runs on host in fp32 to build the ragged schedule: only
positions k < max(n) are computed on device; the rest of the output stays
zero via the pre-zeroed output buffers. keys are computed redundantly on
each device. The big decoder runs with fp16 matmul inputs and fp32 PSUM
accumulation. Per position k: x[:,k,:] = mish((z * keys[k]) @ W1) @ W2,
i.e. scale zT columns-of-keys into the moving matmul operand.

mish(x) = x*tanh(softplus(x)) has no HW activation table; it is computed
exactly via g = sigmoid(-x):  tanh(softplus(x)) = (1-g^2)/(1+g^2), so
  mish(x) = (r - 1) * x   with  r = 2/(1 + g^2)
using one ScalarE pass (Sigmoid), GpSimd passes (g^2, 0.5+0.5*g^2), and DVE
passes (reciprocal, fused (r-1)*x via scalar_tensor_tensor).
"""

import numpy as np
from contextlib import ExitStack

import concourse.bass as bass
import concourse.tile as tile
from concourse import bacc, mybir
from concourse.bass_utils import run_bass_kernel_spmd
from concourse.masks import make_identity

AF = mybir.ActivationFunctionType
ALU = mybir.AluOpType
DT = mybir.dt

B, DIM, HID, MAXN = 2048, 256, 512, 128
MID_S = (HID + 1) // 2      # 256
MID_K = (MAXN + HID) // 2   # 320
MID_D = (HID + DIM) // 2    # 384
NCORES = 8
BS = B // NCORES            # 256 rows per core
HC = HID // 128             # 4 h-chunks
DC = MID_D // 128           # 3 d-chunks
LN_EPS = 1e-5

LAST_RUN = None  # BassKernelResults of the last device launch (for profiling)


# ----------------------------------------------------------------- host math
def _np_mish(x):
    return (x * np.tanh(np.log1p(np.exp(x)))).astype(np.float32)


def _np_ln(x, g, b):
    m = x.mean(-1, keepdims=True, dtype=np.float32)
    v = x.var(-1, keepdims=True, dtype=np.float32)
    return ((x - m) / np.sqrt(v + LN_EPS) * g + b).astype(np.float32)


def _host_size_pred(z, sp_W1, sp_b1, sp_g, sp_be, sp_W2, sp_b2):
    h = _np_mish(_np_ln((z @ sp_W1 + sp_b1).astype(np.float32), sp_g, sp_be))
    nl = (h @ sp_W2 + sp_b2).astype(np.float32)
    return np.clip(np.round(nl[:, 0]), 0, MAXN).astype(np.int32)


# ------------------------------------------------------------- device kernel
def _act_recip(nc, out, in_, scale, bias):
    """ScalarE table reciprocal: out = 1/(in*scale + bias).

    bass.activation() refuses Reciprocal wholesale (generic accuracy
    concerns); our input range is exactly [1, 2] where the 1016-bucket
    table is accurate to ~1e-5 rel (hardware-probed), so emit directly.
    """
    eng = nc.scalar
    ins = [eng.lower_ap(in_)]
    for arg in (bias, scale, 0.0):
        ins.append(mybir.ImmediateValue(dtype=mybir.dt.float32, value=arg))
    return eng.add_instruction(mybir.InstActivation(
        name=nc.get_next_instruction_name(), func=AF.Reciprocal,
        ins=ins, outs=[eng.lower_ap(out)]))


def _emit_mish(nc, pools, frags, parts, width):
    """mish over PSUM fp32 fragments [(x_psum_ap, out16_ap, w), ...].

    mish(x) = (r - 1) * x,  r = 1/(0.5*sigmoid(-x)^2 + 0.5).
    Engine split: Sigmoid on ScalarE (single table set for the whole loop
    -- mixing table sets costs a ~10us reload per switch), g^2 and the
    affine on GpSimd, reciprocal + fused (r-1)*x on DVE.
    """
    g = pools["mg"].tile([parts, width], DT.float32, tag="mg")
    q = pools["mq"].tile([parts, width], DT.float32, tag="mq")
    d = pools["md"].tile([parts, width], DT.float32, tag="md")
    r = pools["mr"].tile([parts, width], DT.float32, tag="mr")
    off = 0
    for x_psum, out16, w in frags:
        gs, qs = g[:, off:off + w], q[:, off:off + w]
        ds, rs = d[:, off:off + w], r[:, off:off + w]
        nc.scalar.activation(gs, x_psum, AF.Sigmoid, scale=-1.0)
        nc.gpsimd.tensor_tensor(qs, gs, gs, ALU.mult)
        nc.gpsimd.tensor_scalar(ds, qs, 0.5, 0.5, ALU.mult, ALU.add)
        nc.vector.reciprocal(rs, ds)
        nc.vector.scalar_tensor_tensor(out16, rs, 1.0, x_psum, ALU.subtract,
                                       ALU.mult)
        off += w


def build_kernel(K, de_b1_nonzero, de_b2_nonzero, kn_b2_nonzero, kn_skip,
                 repeat=1):
    """One SPMD bass program computing x[:, :K, :] for a 256-row shard.

    repeat > 1 wraps the main loop in a hardware For-loop re-running the
    same (idempotent) computation; used only for wall-clock benchmarking.
    """
    nc = bacc.Bacc("TRN2", target_bir_lowering=False, debug=False,
                   num_devices=NCORES)

    z_d = nc.dram_tensor("z", [BS, HID], DT.float32, kind="ExternalInput").ap()
    maskf_d = nc.dram_tensor("maskf", [BS, MAXN], DT.float32,
                             kind="ExternalInput").ap()
    knW1_d = nc.dram_tensor("kn_W1", [MAXN, MID_K], DT.float32,
                            kind="ExternalInput").ap()
    knW2_d = nc.dram_tensor("kn_W2", [MID_K, HID], DT.float32,
                            kind="ExternalInput").ap()
    knb2_d = nc.dram_tensor("kn_b2", [HID], DT.float32, kind="ExternalInput").ap()
    deW1_d = nc.dram_tensor("de_W1", [HID, MID_D], DT.float32,
                            kind="ExternalInput").ap()
    deb1_d = nc.dram_tensor("de_b1", [MID_D], DT.float32, kind="ExternalInput").ap()
    deW2_d = nc.dram_tensor("de_W2", [MID_D, DIM], DT.float32,
                            kind="ExternalInput").ap()
    deb2_d = nc.dram_tensor("de_b2", [DIM], DT.float32, kind="ExternalInput").ap()
    x_d = nc.dram_tensor("x", [BS, MAXN, DIM], DT.float32,
                         kind="ExternalOutput").ap()

    with tile.TileContext(nc) as tc, ExitStack() as ctx:
        wpool = ctx.enter_context(tc.tile_pool(name="weights", bufs=1))
        mish_pools = {nm: ctx.enter_context(tc.tile_pool(name=nm, bufs=2))
                      for nm in ("mg", "mq", "md", "mr")}

        ident = wpool.tile([128, 128], DT.float32, tag="ident")
        make_identity(nc, ident[:])

        with tc.tile_pool(name="psetup", bufs=2, space="PSUM") as psetup, \
             tc.tile_pool(name="setup_sb", bufs=2) as ssb:
            # --- persistent weights (fp16)
            W1h = []   # de_W1, 4 x [128h, 384d]
            for hc in range(HC):
                t32 = ssb.tile([128, MID_D], DT.float32, tag="w1stage")
                nc.sync.dma_start(t32[:], deW1_d[hc * 128:(hc + 1) * 128, :])
                t16 = wpool.tile([128, MID_D], DT.float16, tag=f"w1h{hc}")
                nc.vector.tensor_copy(t16[:], t32[:])
                W1h.append(t16)
            W2h = []   # de_W2, 3 x [128d, 256]
            for dc in range(DC):
                t32 = ssb.tile([128, DIM], DT.float32, tag="w2stage")
                nc.sync.dma_start(t32[:], deW2_d[dc * 128:(dc + 1) * 128, :])
                t16 = wpool.tile([128, DIM], DT.float16, tag=f"w2h{dc}")
                nc.vector.tensor_copy(t16[:], t32[:])
                W2h.append(t16)

            deb1c = []
            if de_b1_nonzero:
                for dc in range(DC):
                    t = wpool.tile([128, 1], DT.float32, tag=f"deb1{dc}")
                    nc.sync.dma_start(t[:], deb1_d[dc * 128:(dc + 1) * 128][:, None])
                    deb1c.append(t)
            deb2row = ones_row = None
            if de_b2_nonzero:
                t32 = ssb.tile([1, DIM], DT.float32, tag="deb2st")
                nc.sync.dma_start(t32[:], deb2_d[None, :])
                deb2row = wpool.tile([1, DIM], DT.float16, tag="deb2h")
                nc.vector.tensor_copy(deb2row[:], t32[:])
                ones_row = wpool.tile([1, 128], DT.float16, tag="ones")
                nc.vector.memset(ones_row[:], 1.0)

            maskf = []  # 2 x [128b, 128k] fp32
            for bt in range(2):
                t = wpool.tile([128, MAXN], DT.float32, tag=f"maskf{bt}")
                nc.sync.dma_start(t[:], maskf_d[bt * 128:(bt + 1) * 128, :])
                maskf.append(t)

            # --- transpose z -> zTh fp16 [128h, 256b] x4
            zTh = [wpool.tile([128, BS], DT.float16, tag=f"zth{hc}", name=f"zth{hc}")
                   for hc in range(HC)]
            for bt in range(2):
                zrow = ssb.tile([128, HID], DT.float32, tag="zrow")
                nc.sync.dma_start(zrow[:], z_d[bt * 128:(bt + 1) * 128, :])
                for hc in range(HC):
                    pt = psetup.tile([128, 128], DT.float32, tag="ptr")
                    nc.tensor.transpose(pt[:], zrow[:, hc * 128:(hc + 1) * 128],
                                        ident[:])
                    nc.scalar.activation(zTh[hc][:, bt * 128:(bt + 1) * 128],
                                         pt[:], AF.Identity)

            # --- keys: km = mish(LN(kn_W1eff))   (kn_b1 folded on host)
            kw1 = ssb.tile([MAXN, MID_K], DT.float32, tag="kw1")
            nc.sync.dma_start(kw1[:], knW1_d)
            if kn_skip:
                km = kw1
            else:
                mean = ssb.tile([128, 1], DT.float32, tag="mean")
                nc.vector.tensor_reduce(mean[:], kw1[:], mybir.AxisListType.X,
                                        ALU.add)
                sq = ssb.tile([MAXN, MID_K], DT.float32, tag="sq")
                ssq = ssb.tile([128, 1], DT.float32, tag="ssq")
                nc.scalar.activation(sq[:], kw1[:], AF.Square, accum_out=ssq[:])
                nc.scalar.mul(mean[:], mean[:], 1.0 / MID_K)
                m2 = ssb.tile([128, 1], DT.float32, tag="m2")
                nc.scalar.activation(m2[:], mean[:], AF.Square)
                var = ssb.tile([128, 1], DT.float32, tag="var")
                nc.vector.scalar_tensor_tensor(var[:], ssq[:], 1.0 / MID_K,
                                               m2[:], ALU.mult, ALU.subtract)
                epsc = ssb.tile([128, 1], DT.float32, tag="epsc")
                nc.gpsimd.memset(epsc[:], LN_EPS)
                srt = ssb.tile([128, 1], DT.float32, tag="srt")
                nc.scalar.activation(srt[:], var[:], AF.Sqrt, bias=epsc[:, 0:1])
                rstd = ssb.tile([128, 1], DT.float32, tag="rstd")
                nc.vector.reciprocal(rstd[:], srt[:])
                nrm = ssb.tile([MAXN, MID_K], DT.float32, tag="nrm")
                nc.vector.tensor_scalar(nrm[:], kw1[:], mean[:, 0:1],
                                        rstd[:, 0:1], ALU.subtract, ALU.mult)
                # mish(nrm) in fp32 (same chain, fp32 output)
                g = mish_pools["mg"].tile([MAXN, MID_K], DT.float32, tag="mg")
                nc.scalar.activation(g[:], nrm[:], AF.Sigmoid, scale=-1.0)
                q = mish_pools["mq"].tile([MAXN, MID_K], DT.float32, tag="mq")
                nc.gpsimd.tensor_tensor(q[:], g[:], g[:], ALU.mult)
                d0 = mish_pools["md"].tile([MAXN, MID_K], DT.float32, tag="md")
                nc.gpsimd.tensor_scalar(d0[:], q[:], 0.5, 0.5, ALU.mult, ALU.add)
                r = mish_pools["mr"].tile([MAXN, MID_K], DT.float32, tag="mr")
                nc.vector.reciprocal(r[:], d0[:])
                km = ssb.tile([MAXN, MID_K], DT.float32, tag="km")
                nc.vector.scalar_tensor_tensor(km[:], r[:], 1.0, nrm[:],
                                               ALU.subtract, ALU.mult)

            # kmT chunks: 320m -> [128,128,64] partitions x [128k]
            mc_sizes = [128, 128, MID_K - 256]
            kmT = []
            for mc, msz in enumerate(mc_sizes):
                pt = psetup.tile([msz, 128], DT.float32, tag="pkmt")
                nc.tensor.transpose(pt[:], km[:, mc * 128:mc * 128 + msz],
                                    ident[:])
                t = ssb.tile([msz, 128], DT.float32, tag=f"kmt{mc}")
                nc.scalar.activation(t[:], pt[:], AF.Identity)
                kmT.append(t)

            knW2t = []
            for mc, msz in enumerate(mc_sizes):
                t = ssb.tile([msz, HID], DT.float32, tag=f"kw2{mc}")
                nc.sync.dma_start(t[:], knW2_d[mc * 128:mc * 128 + msz, :])
                knW2t.append(t)
            keysT = []   # 4 x [128h, 128k] fp32
            for hc in range(HC):
                pk = psetup.tile([128, 128], DT.float32, tag="pkeys")
                for mc, msz in enumerate(mc_sizes):
                    nc.tensor.matmul(pk[:],
                                     knW2t[mc][:, hc * 128:(hc + 1) * 128],
                                     kmT[mc][:], start=(mc == 0), stop=(mc == 2))
                kt = wpool.tile([128, MAXN], DT.float32, tag=f"keysT{hc}")
                if kn_b2_nonzero:
                    kb = ssb.tile([128, 1], DT.float32, tag="knb2c")
                    nc.sync.dma_start(kb[:],
                                      knb2_d[hc * 128:(hc + 1) * 128][:, None])
                    nc.scalar.activation(kt[:], pk[:], AF.Identity,
                                         bias=kb[:, 0:1])
                else:
                    nc.scalar.activation(kt[:], pk[:], AF.Identity)
                keysT.append(kt)

        # ------------------------------------------------------ main k loop
        ps1 = ctx.enter_context(tc.tile_pool(name="ps1", bufs=2, space="PSUM"))
        ps2 = ctx.enter_context(tc.tile_pool(name="ps2", bufs=2, space="PSUM"))
        zkp = ctx.enter_context(tc.tile_pool(name="zk", bufs=3))
        actp = ctx.enter_context(tc.tile_pool(name="act", bufs=2))
        outp = ctx.enter_context(tc.tile_pool(name="outsb", bufs=3))

        def _main_body():
          for k0 in range(0, K, 2):
            nk = min(2, K - k0)
            W = nk * BS
            # zk[hc] = zTh[hc] * keysT[hc][:, k]  (GpSimd, fp16, fp32 scalar)
            zk = [zkp.tile([128, W], DT.float16, tag=f"zk{hc}", name=f"zk{hc}")
                  for hc in range(HC)]
            for kk in range(nk):
                for hc in range(HC):
                    nc.gpsimd.tensor_scalar(
                        zk[hc][:, kk * BS:(kk + 1) * BS], zTh[hc][:],
                        keysT[hc][:, k0 + kk:k0 + kk + 1], None, ALU.mult)
            # mm1 into one fused 3-bank psum tile [128, 3*512]; dc chunks sit
            # at fixed 512-column (one-bank) strides so each accumulation
            # group owns a bank even when nk == 1.
            p1 = ps1.tile([128, DC * 512], DT.float32, tag="p1")
            for dc in range(DC):
                sl = p1[:, dc * 512:dc * 512 + W]
                for hc in range(HC):
                    nc.tensor.matmul(sl, W1h[hc][:, dc * 128:(dc + 1) * 128],
                                     zk[hc][:], start=(hc == 0),
                                     stop=(hc == HC - 1))
                if de_b1_nonzero:
                    nc.vector.tensor_scalar(sl, sl, deb1c[dc][:, 0:1],
                                            None, ALU.add)
            a16 = actp.tile([128, DC * W], DT.float16, tag="a16")
            if nk == 2:
                frags = [(p1[:, :DC * 512], a16[:, :DC * 512], DC * 512)]
            else:
                frags = [(p1[:, dc * 512:dc * 512 + W],
                          a16[:, dc * W:(dc + 1) * W], W) for dc in range(DC)]
            _emit_mish(nc, mish_pools, frags, 128, DC * 512)
            out_sb = [outp.tile([128, W], DT.float32, tag=f"o{bt}", name=f"o{bt}")
                      for bt in range(2)]
            for kk in range(nk):
                for bt in range(2):
                    p2 = ps2.tile([128, DIM], DT.float32, tag="p2")
                    for dc in range(DC):
                        last = (dc == DC - 1) and not de_b2_nonzero
                        nc.tensor.matmul(
                            p2[:],
                            a16[:, dc * W + kk * BS + bt * 128:
                                dc * W + kk * BS + (bt + 1) * 128],
                            W2h[dc][:], start=(dc == 0), stop=last)
                    if de_b2_nonzero:
                        nc.tensor.matmul(p2[:], ones_row[:], deb2row[:],
                                         start=False, stop=True)
                    mcol = maskf[bt][:, k0 + kk:k0 + kk + 1]
                    dst = out_sb[bt][:, kk * DIM:(kk + 1) * DIM]
                    nc.scalar.activation(dst, p2[:], AF.Identity, scale=mcol)
            for bt in range(2):
                nc.sync.dma_start(
                    x_d[bt * 128:(bt + 1) * 128, k0:k0 + nk, :],
                    out_sb[bt][:, :W])

        if repeat > 1:
            with tc.For_i(0, repeat, 1):
                _main_body()
        else:
            _main_body()

    nc.compile()
    return nc


_NEFF_CACHE = {}


def kernel(**inputs):
    global LAST_RUN
    inp = {k: np.ascontiguousarray(np.asarray(v)) for k, v in inputs.items()}
    z = np.ascontiguousarray(inp["z"].astype(np.float32, copy=False))

    # --- host: size head -> n, mask, ragged bound K
    n = _host_size_pred(z, inp["sp_W1"], inp["sp_b1"], inp["sp_g"],
                        inp["sp_be"], inp["sp_W2"], inp["sp_b2"])
    mask = np.arange(MAXN)[None, :] < n[:, None]
    maskf = np.ascontiguousarray(mask.astype(np.float32))
    K = int(n.max())
    x = np.zeros((B, MAXN, DIM), np.float32)
    if K == 0:
        return x, n, mask

    # fold kn_b1 into kn_W1; if LN affine is non-trivial, fold the whole
    # key_net hidden layer on host (device then skips LN+mish).
    kn_w1eff = (inp["kn_W1"] + inp["kn_b1"]).astype(np.float32)
    kn_skip = False
    if not (np.all(inp["kn_g"] == 1.0) and np.all(inp["kn_be"] == 0.0)):
        kn_w1eff = _np_mish(_np_ln(kn_w1eff, inp["kn_g"], inp["kn_be"]))
        kn_skip = True

    de_b1_nonzero = bool(np.any(inp["de_b1"] != 0))
    de_b2_nonzero = bool(np.any(inp["de_b2"] != 0))
    kn_b2_nonzero = bool(np.any(inp["kn_b2"] != 0))

    ck = (K, de_b1_nonzero, de_b2_nonzero, kn_b2_nonzero, kn_skip)
    if ck not in _NEFF_CACHE:
        _NEFF_CACHE[ck] = build_kernel(*ck)
    nc = _NEFF_CACHE[ck]

    f32 = lambda a: np.ascontiguousarray(a.astype(np.float32, copy=False))
    shared = {
        "kn_W1": f32(kn_w1eff),
        "kn_W2": f32(inp["kn_W2"]),
        "kn_b2": f32(inp["kn_b2"]),
        "de_W1": f32(inp["de_W1"]),
        "de_b1": f32(inp["de_b1"]),
        "de_W2": f32(inp["de_W2"]),
        "de_b2": f32(inp["de_b2"]),
    }
    in_maps = [{**shared, "z": z[c * BS:(c + 1) * BS],
                "maskf": maskf[c * BS:(c + 1) * BS]} for c in range(NCORES)]

    LAST_RUN = run_bass_kernel_spmd(nc, in_maps, list(range(NCORES)))
    for c in range(NCORES):
        x[c * BS:(c + 1) * BS] = np.asarray(LAST_RUN.results[c]["x"])
    return x, n, mask


# revision 14
# speedup vs baseline: 1.8053x; 1.4911x over previous
"""Trainium2 Bass kernel for nn_Decoder (ragged sequence decoder).

Reference math:
  n      = clip(round(size_pred(z)), 0, 128)            [B]  (tiny scalar head)
  keys   = mish(LN(kn_W1 + kn_b1)) @ kn_W2 + kn_b2      [128, 512]
  x      = mish((z[:,None,:] * keys[None]) @ de_W1 + de_b1) @ de_W2 + de_b2
  x     *= (arange(128) < n[:,None])[..., None]         zero padded slots

Strategy: pure data parallel over batch (8 cores x 256 rows). The size head
(0.3% of FLOPs) runs on host in fp32 to build the ragged schedule: only
positions k < max(n) are computed on device; the rest of the output stays
zero via the pre-zeroed output buffers. keys are computed redundantly on
each device. The big decoder runs with fp16 matmul inputs and fp32 PSUM
accumulation. Per position k: x[:,k,:] = mish((z * keys[k]) @ W1) @ W2,
i.e. scale zT columns-of-keys into the moving matmul operand.

mish(x) = x*tanh(softplus(x)) has no HW activation table; it is computed
exactly via g = sigmoid(-x):  tanh(softplus(x)) = (1-g^2)/(1+g^2), so
  mish(x) = (r - 1) * x   with  r = 2/(1 + g^2)
using one ScalarE pass (Sigmoid), GpSimd passes (g^2, 0.5+0.5*g^2), and DVE
passes (reciprocal, fused (r-1)*x via scalar_tensor_tensor).
"""

import numpy as np
from contextlib import ExitStack

import concourse.bass as bass
import concourse.tile as tile
from concourse import bacc, mybir
from concourse.bass_utils import run_bass_kernel_spmd
from concourse.masks import make_identity

AF = mybir.ActivationFunctionType
ALU = mybir.AluOpType
DT = mybir.dt

B, DIM, HID, MAXN = 2048, 256, 512, 128
MID_S = (HID + 1) // 2      # 256
MID_K = (MAXN + HID) // 2   # 320
MID_D = (HID + DIM) // 2    # 384
NCORES = 8
BS = B // NCORES            # 256 rows per core
HC = HID // 128             # 4 h-chunks
DC = MID_D // 128           # 3 d-chunks
LN_EPS = 1e-5

LAST_RUN = None  # BassKernelResults of the last device launch (for profiling)


# ----------------------------------------------------------------- host math
def _np_mish(x):
    return (x * np.tanh(np.log1p(np.exp(x)))).astype(np.float32)


def _np_ln(x, g, b):
    m = x.mean(-1, keepdims=True, dtype=np.float32)
    v = x.var(-1, keepdims=True, dtype=np.float32)
    return ((x - m) / np.sqrt(v + LN_EPS) * g + b).astype(np.float32)


def _host_size_pred(z, sp_W1, sp_b1, sp_g, sp_be, sp_W2, sp_b2):
    h = _np_mish(_np_ln((z @ sp_W1 + sp_b1).astype(np.float32), sp_g, sp_be))
    nl = (h @ sp_W2 + sp_b2).astype(np.float32)
    return np.clip(np.round(nl[:, 0]), 0, MAXN).astype(np.int32)


# ------------------------------------------------------------- device kernel
def _act_recip(nc, out, in_, scale, bias):
    """ScalarE table reciprocal: out = 1/(in*scale + bias).

    bass.activation() refuses Reciprocal wholesale (generic accuracy
    concerns); our input range is exactly [1, 2] where the 1016-bucket
    table is accurate to ~1e-5 rel (hardware-probed), so emit directly.
    """
    eng = nc.scalar
    ins = [eng.lower_ap(in_)]
    for arg in (bias, scale, 0.0):
        ins.append(mybir.ImmediateValue(dtype=mybir.dt.float32, value=arg))
    return eng.add_instruction(mybir.InstActivation(
        name=nc.get_next_instruction_name(), func=AF.Reciprocal,
        ins=ins, outs=[eng.lower_ap(out)]))


def _emit_mish(nc, pools, frags, parts, width):
    """mish over PSUM fp32 fragments [(x_psum_ap, out16_ap, w), ...].

    mish(x) = (r - 1) * x,  r = 1/(0.5*sigmoid(-x)^2 + 0.5).
    Engine split: Sigmoid on ScalarE (single table set for the whole loop
    -- mixing table sets costs a ~10us reload per switch), g^2 and the
    affine on GpSimd, reciprocal + fused (r-1)*x on DVE.
    """
    g = pools["mg"].tile([parts, width], DT.float32, tag="mg")
    q = pools["mq"].tile([parts, width], DT.float32, tag="mq")
    d = pools["md"].tile([parts, width], DT.float32, tag="md")
    r = pools["mr"].tile([parts, width], DT.float32, tag="mr")
    off = 0
    for x_psum, out16, w in frags:
        gs, qs = g[:, off:off + w], q[:, off:off + w]
        ds, rs = d[:, off:off + w], r[:, off:off + w]
        nc.scalar.activation(gs, x_psum, AF.Sigmoid, scale=-1.0)
        nc.gpsimd.tensor_tensor(qs, gs, gs, ALU.mult)
        nc.gpsimd.tensor_scalar(ds, qs, 0.5, 0.5, ALU.mult, ALU.add)
        nc.vector.reciprocal(rs, ds)
        nc.vector.scalar_tensor_tensor(out16, rs, 1.0, x_psum, ALU.subtract,
                                       ALU.mult)
        off += w


def build_kernel(K, de_b1_nonzero, de_b2_nonzero, kn_b2_nonzero, kn_skip,
                 repeat=1):
    """One SPMD bass program computing x[:, :K, :] for a 256-row shard.

    repeat > 1 wraps the main loop in a hardware For-loop re-running the
    same (idempotent) computation; used only for wall-clock benchmarking.
    """
    nc = bacc.Bacc("TRN2", target_bir_lowering=False, debug=False,
                   num_devices=NCORES)

    z_d = nc.dram_tensor("z", [BS, HID], DT.float32, kind="ExternalInput").ap()
    maskf_d = nc.dram_tensor("maskf", [BS, MAXN], DT.float32,
                             kind="ExternalInput").ap()
    knW1_d = nc.dram_tensor("kn_W1", [MAXN, MID_K], DT.float32,
                            kind="ExternalInput").ap()
    knW2_d = nc.dram_tensor("kn_W2", [MID_K, HID], DT.float32,
                            kind="ExternalInput").ap()
    knb2_d = nc.dram_tensor("kn_b2", [HID], DT.float32, kind="ExternalInput").ap()
    deW1_d = nc.dram_tensor("de_W1", [HID, MID_D], DT.float32,
                            kind="ExternalInput").ap()
    deb1_d = nc.dram_tensor("de_b1", [MID_D], DT.float32, kind="ExternalInput").ap()
    deW2_d = nc.dram_tensor("de_W2", [MID_D, DIM], DT.float32,
                            kind="ExternalInput").ap()
    deb2_d = nc.dram_tensor("de_b2", [DIM], DT.float32, kind="ExternalInput").ap()
    x_d = nc.dram_tensor("x", [BS, MAXN, DIM], DT.float32,
                         kind="ExternalOutput").ap()

    with tile.TileContext(nc) as tc, ExitStack() as ctx:
        wpool = ctx.enter_context(tc.tile_pool(name="weights", bufs=1))
        mish_pools = {nm: ctx.enter_context(tc.tile_pool(name=nm, bufs=2))
                      for nm in ("mg", "mq", "md", "mr")}

        ident = wpool.tile([128, 128], DT.float32, tag="ident")
        make_identity(nc, ident[:])

        with tc.tile_pool(name="psetup", bufs=2, space="PSUM") as psetup, \
             tc.tile_pool(name="setup_sb", bufs=2) as ssb:
            # --- persistent weights (fp16)
            W1h = []   # de_W1, 4 x [128h, 384d]
            for hc in range(HC):
                t32 = ssb.tile([128, MID_D], DT.float32, tag="w1stage")
                nc.sync.dma_start(t32[:], deW1_d[hc * 128:(hc + 1) * 128, :])
                t16 = wpool.tile([128, MID_D], DT.float16, tag=f"w1h{hc}")
                nc.vector.tensor_copy(t16[:], t32[:])
                W1h.append(t16)
            W2h = []   # de_W2, 3 x [128d, 256]
            for dc in range(DC):
                t32 = ssb.tile([128, DIM], DT.float32, tag="w2stage")
                nc.sync.dma_start(t32[:], deW2_d[dc * 128:(dc + 1) * 128, :])
                t16 = wpool.tile([128, DIM], DT.float16, tag=f"w2h{dc}")
                nc.vector.tensor_copy(t16[:], t32[:])
                W2h.append(t16)

            deb1c = []
            if de_b1_nonzero:
                for dc in range(DC):
                    t = wpool.tile([128, 1], DT.float32, tag=f"deb1{dc}")
                    nc.sync.dma_start(t[:], deb1_d[dc * 128:(dc + 1) * 128][:, None])
                    deb1c.append(t)
            deb2row = ones_row = None
            if de_b2_nonzero:
                t32 = ssb.tile([1, DIM], DT.float32, tag="deb2st")
                nc.sync.dma_start(t32[:], deb2_d[None, :])
                deb2row = wpool.tile([1, DIM], DT.float16, tag="deb2h")
                nc.vector.tensor_copy(deb2row[:], t32[:])
                ones_row = wpool.tile([1, 128], DT.float16, tag="ones")
                nc.vector.memset(ones_row[:], 1.0)

            maskf = []  # 2 x [128b, 128k] fp32
            for bt in range(2):
                t = wpool.tile([128, MAXN], DT.float32, tag=f"maskf{bt}")
                nc.sync.dma_start(t[:], maskf_d[bt * 128:(bt + 1) * 128, :])
                maskf.append(t)

            # --- transpose z -> zTh fp16 [128h, 256b] x4
            zTh = [wpool.tile([128, BS], DT.float16, tag=f"zth{hc}", name=f"zth{hc}")
                   for hc in range(HC)]
            for bt in range(2):
                zrow = ssb.tile([128, HID], DT.float32, tag="zrow")
                nc.sync.dma_start(zrow[:], z_d[bt * 128:(bt + 1) * 128, :])
                for hc in range(HC):
                    pt = psetup.tile([128, 128], DT.float32, tag="ptr")
                    nc.tensor.transpose(pt[:], zrow[:, hc * 128:(hc + 1) * 128],
                                        ident[:])
                    nc.scalar.activation(zTh[hc][:, bt * 128:(bt + 1) * 128],
                                         pt[:], AF.Identity)

            # --- keys: km = mish(LN(kn_W1eff))   (kn_b1 folded on host)
            kw1 = ssb.tile([MAXN, MID_K], DT.float32, tag="kw1")
            nc.sync.dma_start(kw1[:], knW1_d)
            if kn_skip:
                km = kw1
            else:
                mean = ssb.tile([128, 1], DT.float32, tag="mean")
                nc.vector.tensor_reduce(mean[:], kw1[:], mybir.AxisListType.X,
                                        ALU.add)
                sq = ssb.tile([MAXN, MID_K], DT.float32, tag="sq")
                ssq = ssb.tile([128, 1], DT.float32, tag="ssq")
                nc.scalar.activation(sq[:], kw1[:], AF.Square, accum_out=ssq[:])
                nc.scalar.mul(mean[:], mean[:], 1.0 / MID_K)
                m2 = ssb.tile([128, 1], DT.float32, tag="m2")
                nc.scalar.activation(m2[:], mean[:], AF.Square)
                var = ssb.tile([128, 1], DT.float32, tag="var")
                nc.vector.scalar_tensor_tensor(var[:], ssq[:], 1.0 / MID_K,
                                               m2[:], ALU.mult, ALU.subtract)
                epsc = ssb.tile([128, 1], DT.float32, tag="epsc")
                nc.gpsimd.memset(epsc[:], LN_EPS)
                srt = ssb.tile([128, 1], DT.float32, tag="srt")
                nc.scalar.activation(srt[:], var[:], AF.Sqrt, bias=epsc[:, 0:1])
                rstd = ssb.tile([128, 1], DT.float32, tag="rstd")
                nc.vector.reciprocal(rstd[:], srt[:])
                nrm = ssb.tile([MAXN, MID_K], DT.float32, tag="nrm")
                nc.vector.tensor_scalar(nrm[:], kw1[:], mean[:, 0:1],
                                        rstd[:, 0:1], ALU.subtract, ALU.mult)
                # mish(nrm) in fp32 (same chain, fp32 output)
                g = mish_pools["mg"].tile([MAXN, MID_K], DT.float32, tag="mg")
                nc.scalar.activation(g[:], nrm[:], AF.Sigmoid, scale=-1.0)
                q = mish_pools["mq"].tile([MAXN, MID_K], DT.float32, tag="mq")
                nc.gpsimd.tensor_tensor(q[:], g[:], g[:], ALU.mult)
                d0 = mish_pools["md"].tile([MAXN, MID_K], DT.float32, tag="md")
                nc.gpsimd.tensor_scalar(d0[:], q[:], 0.5, 0.5, ALU.mult, ALU.add)
                r = mish_pools["mr"].tile([MAXN, MID_K], DT.float32, tag="mr")
                nc.vector.reciprocal(r[:], d0[:])
                km = ssb.tile([MAXN, MID_K], DT.float32, tag="km")
                nc.vector.scalar_tensor_tensor(km[:], r[:], 1.0, nrm[:],
                                               ALU.subtract, ALU.mult)

            # kmT chunks: 320m -> [128,128,64] partitions x [128k]
            mc_sizes = [128, 128, MID_K - 256]
            kmT = []
            for mc, msz in enumerate(mc_sizes):
                pt = psetup.tile([msz, 128], DT.float32, tag="pkmt")
                nc.tensor.transpose(pt[:], km[:, mc * 128:mc * 128 + msz],
                                    ident[:])
                t = ssb.tile([msz, 128], DT.float32, tag=f"kmt{mc}")
                nc.scalar.activation(t[:], pt[:], AF.Identity)
                kmT.append(t)

            knW2t = []
            for mc, msz in enumerate(mc_sizes):
                t = ssb.tile([msz, HID], DT.float32, tag=f"kw2{mc}")
                nc.sync.dma_start(t[:], knW2_d[mc * 128:mc * 128 + msz, :])
                knW2t.append(t)
            keysT = []   # 4 x [128h, 128k] fp32
            for hc in range(HC):
                pk = psetup.tile([128, 128], DT.float32, tag="pkeys")
                for mc, msz in enumerate(mc_sizes):
                    nc.tensor.matmul(pk[:],
                                     knW2t[mc][:, hc * 128:(hc + 1) * 128],
                                     kmT[mc][:], start=(mc == 0), stop=(mc == 2))
                kt = wpool.tile([128, MAXN], DT.float32, tag=f"keysT{hc}")
                if kn_b2_nonzero:
                    kb = ssb.tile([128, 1], DT.float32, tag="knb2c")
                    nc.sync.dma_start(kb[:],
                                      knb2_d[hc * 128:(hc + 1) * 128][:, None])
                    nc.scalar.activation(kt[:], pk[:], AF.Identity,
                                         bias=kb[:, 0:1])
                else:
                    nc.scalar.activation(kt[:], pk[:], AF.Identity)
                keysT.append(kt)

        # ------------------------------------------------------ main k loop
        ps1 = ctx.enter_context(tc.tile_pool(name="ps1", bufs=2, space="PSUM"))
        ps2 = ctx.enter_context(tc.tile_pool(name="ps2", bufs=2, space="PSUM"))
        zkp = ctx.enter_context(tc.tile_pool(name="zk", bufs=3))
        actp = ctx.enter_context(tc.tile_pool(name="act", bufs=2))
        outp = ctx.enter_context(tc.tile_pool(name="outsb", bufs=3))

        def _main_body():
          for k0 in range(0, K, 2):
            nk = min(2, K - k0)
            W = nk * BS
            # zk[hc] = zTh[hc] * keysT[hc][:, k]  (GpSimd, fp16, fp32 scalar)
            zk = [zkp.tile([128, W], DT.float16, tag=f"zk{hc}", name=f"zk{hc}")
                  for hc in range(HC)]
            for kk in range(nk):
                for hc in range(HC):
                    nc.vector.tensor_scalar(
                        zk[hc][:, kk * BS:(kk + 1) * BS], zTh[hc][:],
                        keysT[hc][:, k0 + kk:k0 + kk + 1], None, ALU.mult)
            # mm1 into one fused 3-bank psum tile [128, 3*512]; dc chunks sit
            # at fixed 512-column (one-bank) strides so each accumulation
            # group owns a bank even when nk == 1.
            p1 = ps1.tile([128, DC * 512], DT.float32, tag="p1")
            for dc in range(DC):
                sl = p1[:, dc * 512:dc * 512 + W]
                for hc in range(HC):
                    nc.tensor.matmul(sl, W1h[hc][:, dc * 128:(dc + 1) * 128],
                                     zk[hc][:], start=(hc == 0),
                                     stop=(hc == HC - 1))
                if de_b1_nonzero:
                    nc.vector.tensor_scalar(sl, sl, deb1c[dc][:, 0:1],
                                            None, ALU.add)
            a16 = actp.tile([128, DC * W], DT.float16, tag="a16")
            if nk == 2:
                frags = [(p1[:, :DC * 512], a16[:, :DC * 512], DC * 512)]
            else:
                frags = [(p1[:, dc * 512:dc * 512 + W],
                          a16[:, dc * W:(dc + 1) * W], W) for dc in range(DC)]
            _emit_mish(nc, mish_pools, frags, 128, DC * 512)
            out_sb = [outp.tile([128, W], DT.float32, tag=f"o{bt}", name=f"o{bt}")
                      for bt in range(2)]
            for kk in range(nk):
                for bt in range(2):
                    p2 = ps2.tile([128, DIM], DT.float32, tag="p2")
                    for dc in range(DC):
                        last = (dc == DC - 1) and not de_b2_nonzero
                        nc.tensor.matmul(
                            p2[:],
                            a16[:, dc * W + kk * BS + bt * 128:
                                dc * W + kk * BS + (bt + 1) * 128],
                            W2h[dc][:], start=(dc == 0), stop=last)
                    if de_b2_nonzero:
                        nc.tensor.matmul(p2[:], ones_row[:], deb2row[:],
                                         start=False, stop=True)
                    mcol = maskf[bt][:, k0 + kk:k0 + kk + 1]
                    dst = out_sb[bt][:, kk * DIM:(kk + 1) * DIM]
                    nc.scalar.activation(dst, p2[:], AF.Identity, scale=mcol)
            for bt in range(2):
                nc.sync.dma_start(
                    x_d[bt * 128:(bt + 1) * 128, k0:k0 + nk, :],
                    out_sb[bt][:, :W])

        if repeat > 1:
            with tc.For_i(0, repeat, 1):
                _main_body()
        else:
            _main_body()

    nc.compile()
    return nc


_NEFF_CACHE = {}


def kernel(**inputs):
    global LAST_RUN
    inp = {k: np.ascontiguousarray(np.asarray(v)) for k, v in inputs.items()}
    z = np.ascontiguousarray(inp["z"].astype(np.float32, copy=False))

    # --- host: size head -> n, mask, ragged bound K
    n = _host_size_pred(z, inp["sp_W1"], inp["sp_b1"], inp["sp_g"],
                        inp["sp_be"], inp["sp_W2"], inp["sp_b2"])
    mask = np.arange(MAXN)[None, :] < n[:, None]
    maskf = np.ascontiguousarray(mask.astype(np.float32))
    K = int(n.max())
    x = np.zeros((B, MAXN, DIM), np.float32)
    if K == 0:
        return x, n, mask

    # fold kn_b1 into kn_W1; if LN affine is non-trivial, fold the whole
    # key_net hidden layer on host (device then skips LN+mish).
    kn_w1eff = (inp["kn_W1"] + inp["kn_b1"]).astype(np.float32)
    kn_skip = False
    if not (np.all(inp["kn_g"] == 1.0) and np.all(inp["kn_be"] == 0.0)):
        kn_w1eff = _np_mish(_np_ln(kn_w1eff, inp["kn_g"], inp["kn_be"]))
        kn_skip = True

    de_b1_nonzero = bool(np.any(inp["de_b1"] != 0))
    de_b2_nonzero = bool(np.any(inp["de_b2"] != 0))
    kn_b2_nonzero = bool(np.any(inp["kn_b2"] != 0))

    ck = (K, de_b1_nonzero, de_b2_nonzero, kn_b2_nonzero, kn_skip)
    if ck not in _NEFF_CACHE:
        _NEFF_CACHE[ck] = build_kernel(*ck)
    nc = _NEFF_CACHE[ck]

    f32 = lambda a: np.ascontiguousarray(a.astype(np.float32, copy=False))
    shared = {
        "kn_W1": f32(kn_w1eff),
        "kn_W2": f32(inp["kn_W2"]),
        "kn_b2": f32(inp["kn_b2"]),
        "de_W1": f32(inp["de_W1"]),
        "de_b1": f32(inp["de_b1"]),
        "de_W2": f32(inp["de_W2"]),
        "de_b2": f32(inp["de_b2"]),
    }
    in_maps = [{**shared, "z": z[c * BS:(c + 1) * BS],
                "maskf": maskf[c * BS:(c + 1) * BS]} for c in range(NCORES)]

    LAST_RUN = run_bass_kernel_spmd(nc, in_maps, list(range(NCORES)))
    for c in range(NCORES):
        x[c * BS:(c + 1) * BS] = np.asarray(LAST_RUN.results[c]["x"])
    return x, n, mask


# revision 15
# speedup vs baseline: 3.0936x; 1.7136x over previous
"""Trainium2 Bass kernel for nn_Decoder (ragged sequence decoder).

Reference math:
  n      = clip(round(size_pred(z)), 0, 128)            [B]  (tiny scalar head)
  keys   = mish(LN(kn_W1 + kn_b1)) @ kn_W2 + kn_b2      [128, 512]
  x      = mish((z[:,None,:] * keys[None]) @ de_W1 + de_b1) @ de_W2 + de_b2
  x     *= (arange(128) < n[:,None])[..., None]         zero padded slots

Strategy: pure data parallel over batch (8 cores x 256 rows). The size head
(0.3% of FLOPs) runs on host in fp32 to build the ragged schedule: only
positions k < max(n) are computed on device; the rest of the output stays
zero via the pre-zeroed output buffers. keys are computed redundantly on
each device. The big decoder runs with fp16 matmul inputs and fp32 PSUM
accumulation. Per position k: x[:,k,:] = mish((z * keys[k]) @ W1) @ W2,
i.e. scale zT columns-of-keys into the moving matmul operand.

mish(x) = x*tanh(softplus(x)) has no HW activation table; it is computed
exactly via g = sigmoid(-x):  tanh(softplus(x)) = (1-g^2)/(1+g^2), so
  mish(x) = (r - 1) * x   with  r = 2/(1 + g^2)
using one ScalarE pass (Sigmoid), GpSimd passes (g^2, 0.5+0.5*g^2), and DVE
passes (reciprocal, fused (r-1)*x via scalar_tensor_tensor).
"""

import numpy as np
from contextlib import ExitStack

import concourse.bass as bass
import concourse.tile as tile
from concourse import bacc, mybir
from concourse.bass_utils import run_bass_kernel_spmd
from concourse.masks import make_identity

AF = mybir.ActivationFunctionType
ALU = mybir.AluOpType
DT = mybir.dt

B, DIM, HID, MAXN = 2048, 256, 512, 128
MID_S = (HID + 1) // 2      # 256
MID_K = (MAXN + HID) // 2   # 320
MID_D = (HID + DIM) // 2    # 384
NCORES = 8
BS = B // NCORES            # 256 rows per core
HC = HID // 128             # 4 h-chunks
DC = MID_D // 128           # 3 d-chunks
LN_EPS = 1e-5

LAST_RUN = None  # BassKernelResults of the last device launch (for profiling)


# ----------------------------------------------------------------- host math
def _np_mish(x):
    return (x * np.tanh(np.log1p(np.exp(x)))).astype(np.float32)


def _np_ln(x, g, b):
    m = x.mean(-1, keepdims=True, dtype=np.float32)
    v = x.var(-1, keepdims=True, dtype=np.float32)
    return ((x - m) / np.sqrt(v + LN_EPS) * g + b).astype(np.float32)


def _host_size_pred(z, sp_W1, sp_b1, sp_g, sp_be, sp_W2, sp_b2):
    h = _np_mish(_np_ln((z @ sp_W1 + sp_b1).astype(np.float32), sp_g, sp_be))
    nl = (h @ sp_W2 + sp_b2).astype(np.float32)
    return np.clip(np.round(nl[:, 0]), 0, MAXN).astype(np.int32)


# ------------------------------------------------------------- device kernel
def _act_recip(nc, out, in_, scale, bias):
    """ScalarE table reciprocal: out = 1/(in*scale + bias).

    bass.activation() refuses Reciprocal wholesale (generic accuracy
    concerns); our input range is exactly [1, 2] where the 1016-bucket
    table is accurate to ~1e-5 rel (hardware-probed), so emit directly.
    """
    eng = nc.scalar
    ins = [eng.lower_ap(in_)]
    for arg in (bias, scale, 0.0):
        ins.append(mybir.ImmediateValue(dtype=mybir.dt.float32, value=arg))
    return eng.add_instruction(mybir.InstActivation(
        name=nc.get_next_instruction_name(), func=AF.Reciprocal,
        ins=ins, outs=[eng.lower_ap(out)]))


def _emit_mish(nc, pools, frags, parts, width):
    """mish over PSUM fp32 fragments [(x_psum_ap, out16_ap, w), ...].

    mish(x) = (r - 1) * x,  r = 1/(0.5*sigmoid(-x)^2 + 0.5).
    Engine split: Sigmoid on ScalarE (single table set for the whole loop
    -- mixing table sets costs a ~10us reload per switch), g^2 and the
    affine on GpSimd, reciprocal + fused (r-1)*x on DVE.
    """
    g = pools["mg"].tile([parts, width], DT.float32, tag="mg")
    q = pools["mq"].tile([parts, width], DT.float32, tag="mq")
    d = pools["md"].tile([parts, width], DT.float32, tag="md")
    r = pools["mr"].tile([parts, width], DT.float32, tag="mr")
    off = 0
    for x_psum, out16, w in frags:
        gs, qs = g[:, off:off + w], q[:, off:off + w]
        ds, rs = d[:, off:off + w], r[:, off:off + w]
        nc.scalar.activation(gs, x_psum, AF.Sigmoid, scale=-1.0)
        nc.gpsimd.tensor_tensor(qs, gs, gs, ALU.mult)
        nc.gpsimd.tensor_scalar(ds, qs, 0.5, 0.5, ALU.mult, ALU.add)
        nc.vector.reciprocal(rs, ds)
        nc.vector.scalar_tensor_tensor(out16, rs, 1.0, x_psum, ALU.subtract,
                                       ALU.mult)
        off += w


def build_kernel(K, de_b1_nonzero, de_b2_nonzero, kn_b2_nonzero, kn_skip,
                 repeat=1):
    """One SPMD bass program computing x[:, :K, :] for a 256-row shard.

    repeat > 1 wraps the main loop in a hardware For-loop re-running the
    same (idempotent) computation; used only for wall-clock benchmarking.
    """
    nc = bacc.Bacc("TRN2", target_bir_lowering=False, debug=False,
                   num_devices=NCORES)

    z_d = nc.dram_tensor("z", [BS, HID], DT.float32, kind="ExternalInput").ap()
    maskf_d = nc.dram_tensor("maskf", [BS, MAXN], DT.float32,
                             kind="ExternalInput").ap()
    knW1_d = nc.dram_tensor("kn_W1", [MAXN, MID_K], DT.float32,
                            kind="ExternalInput").ap()
    knW2_d = nc.dram_tensor("kn_W2", [MID_K, HID], DT.float32,
                            kind="ExternalInput").ap()
    knb2_d = nc.dram_tensor("kn_b2", [HID], DT.float32, kind="ExternalInput").ap()
    deW1_d = nc.dram_tensor("de_W1", [HID, MID_D], DT.float32,
                            kind="ExternalInput").ap()
    deb1_d = nc.dram_tensor("de_b1", [MID_D], DT.float32, kind="ExternalInput").ap()
    deW2_d = nc.dram_tensor("de_W2", [MID_D, DIM], DT.float32,
                            kind="ExternalInput").ap()
    deb2_d = nc.dram_tensor("de_b2", [DIM], DT.float32, kind="ExternalInput").ap()
    x_d = nc.dram_tensor("x", [BS, MAXN, DIM], DT.float32,
                         kind="ExternalOutput").ap()

    with tile.TileContext(nc) as tc, ExitStack() as ctx:
        wpool = ctx.enter_context(tc.tile_pool(name="weights", bufs=1))
        mish_pools = {nm: ctx.enter_context(tc.tile_pool(name=nm, bufs=2))
                      for nm in ("mg", "mq", "md", "mr")}

        ident = wpool.tile([128, 128], DT.float32, tag="ident")
        make_identity(nc, ident[:])

        with tc.tile_pool(name="psetup", bufs=2, space="PSUM") as psetup, \
             tc.tile_pool(name="setup_sb", bufs=2) as ssb:
            # --- persistent weights (fp16)
            W1h = []   # de_W1, 4 x [128h, 384d]
            for hc in range(HC):
                t32 = ssb.tile([128, MID_D], DT.float32, tag="w1stage")
                nc.sync.dma_start(t32[:], deW1_d[hc * 128:(hc + 1) * 128, :])
                t16 = wpool.tile([128, MID_D], DT.float16, tag=f"w1h{hc}")
                nc.vector.tensor_copy(t16[:], t32[:])
                W1h.append(t16)
            W2h = []   # de_W2, 3 x [128d, 256]
            for dc in range(DC):
                t32 = ssb.tile([128, DIM], DT.float32, tag="w2stage")
                nc.sync.dma_start(t32[:], deW2_d[dc * 128:(dc + 1) * 128, :])
                t16 = wpool.tile([128, DIM], DT.float16, tag=f"w2h{dc}")
                nc.vector.tensor_copy(t16[:], t32[:])
                W2h.append(t16)

            deb1c = []
            if de_b1_nonzero:
                for dc in range(DC):
                    t = wpool.tile([128, 1], DT.float32, tag=f"deb1{dc}")
                    nc.sync.dma_start(t[:], deb1_d[dc * 128:(dc + 1) * 128][:, None])
                    deb1c.append(t)
            deb2row = ones_row = None
            if de_b2_nonzero:
                t32 = ssb.tile([1, DIM], DT.float32, tag="deb2st")
                nc.sync.dma_start(t32[:], deb2_d[None, :])
                deb2row = wpool.tile([1, DIM], DT.float16, tag="deb2h")
                nc.vector.tensor_copy(deb2row[:], t32[:])
                ones_row = wpool.tile([1, 128], DT.float16, tag="ones")
                nc.vector.memset(ones_row[:], 1.0)

            maskf = []  # 2 x [128b, 128k] fp32
            for bt in range(2):
                t = wpool.tile([128, MAXN], DT.float32, tag=f"maskf{bt}")
                nc.sync.dma_start(t[:], maskf_d[bt * 128:(bt + 1) * 128, :])
                maskf.append(t)

            # --- transpose z -> zTh fp16 [128h, 256b] x4
            zTh = [wpool.tile([128, BS], DT.float16, tag=f"zth{hc}", name=f"zth{hc}")
                   for hc in range(HC)]
            for bt in range(2):
                zrow = ssb.tile([128, HID], DT.float32, tag="zrow")
                nc.sync.dma_start(zrow[:], z_d[bt * 128:(bt + 1) * 128, :])
                for hc in range(HC):
                    pt = psetup.tile([128, 128], DT.float32, tag="ptr")
                    nc.tensor.transpose(pt[:], zrow[:, hc * 128:(hc + 1) * 128],
                                        ident[:])
                    nc.scalar.activation(zTh[hc][:, bt * 128:(bt + 1) * 128],
                                         pt[:], AF.Identity)

            # --- keys: km = mish(LN(kn_W1eff))   (kn_b1 folded on host)
            kw1 = ssb.tile([MAXN, MID_K], DT.float32, tag="kw1")
            nc.sync.dma_start(kw1[:], knW1_d)
            if kn_skip:
                km = kw1
            else:
                mean = ssb.tile([128, 1], DT.float32, tag="mean")
                nc.vector.tensor_reduce(mean[:], kw1[:], mybir.AxisListType.X,
                                        ALU.add)
                sq = ssb.tile([MAXN, MID_K], DT.float32, tag="sq")
                ssq = ssb.tile([128, 1], DT.float32, tag="ssq")
                nc.scalar.activation(sq[:], kw1[:], AF.Square, accum_out=ssq[:])
                nc.scalar.mul(mean[:], mean[:], 1.0 / MID_K)
                m2 = ssb.tile([128, 1], DT.float32, tag="m2")
                nc.scalar.activation(m2[:], mean[:], AF.Square)
                var = ssb.tile([128, 1], DT.float32, tag="var")
                nc.vector.scalar_tensor_tensor(var[:], ssq[:], 1.0 / MID_K,
                                               m2[:], ALU.mult, ALU.subtract)
                epsc = ssb.tile([128, 1], DT.float32, tag="epsc")
                nc.gpsimd.memset(epsc[:], LN_EPS)
                srt = ssb.tile([128, 1], DT.float32, tag="srt")
                nc.scalar.activation(srt[:], var[:], AF.Sqrt, bias=epsc[:, 0:1])
                rstd = ssb.tile([128, 1], DT.float32, tag="rstd")
                nc.vector.reciprocal(rstd[:], srt[:])
                nrm = ssb.tile([MAXN, MID_K], DT.float32, tag="nrm")
                nc.vector.tensor_scalar(nrm[:], kw1[:], mean[:, 0:1],
                                        rstd[:, 0:1], ALU.subtract, ALU.mult)
                # mish(nrm) in fp32 (same chain, fp32 output)
                g = mish_pools["mg"].tile([MAXN, MID_K], DT.float32, tag="mg")
                nc.scalar.activation(g[:], nrm[:], AF.Sigmoid, scale=-1.0)
                q = mish_pools["mq"].tile([MAXN, MID_K], DT.float32, tag="mq")
                nc.gpsimd.tensor_tensor(q[:], g[:], g[:], ALU.mult)
                d0 = mish_pools["md"].tile([MAXN, MID_K], DT.float32, tag="md")
                nc.gpsimd.tensor_scalar(d0[:], q[:], 0.5, 0.5, ALU.mult, ALU.add)
                r = mish_pools["mr"].tile([MAXN, MID_K], DT.float32, tag="mr")
                nc.vector.reciprocal(r[:], d0[:])
                km = ssb.tile([MAXN, MID_K], DT.float32, tag="km")
                nc.vector.scalar_tensor_tensor(km[:], r[:], 1.0, nrm[:],
                                               ALU.subtract, ALU.mult)

            # kmT chunks: 320m -> [128,128,64] partitions x [128k]
            mc_sizes = [128, 128, MID_K - 256]
            kmT = []
            for mc, msz in enumerate(mc_sizes):
                pt = psetup.tile([msz, 128], DT.float32, tag="pkmt")
                nc.tensor.transpose(pt[:], km[:, mc * 128:mc * 128 + msz],
                                    ident[:])
                t = ssb.tile([msz, 128], DT.float32, tag=f"kmt{mc}")
                nc.scalar.activation(t[:], pt[:], AF.Identity)
                kmT.append(t)

            knW2t = []
            for mc, msz in enumerate(mc_sizes):
                t = ssb.tile([msz, HID], DT.float32, tag=f"kw2{mc}")
                nc.sync.dma_start(t[:], knW2_d[mc * 128:mc * 128 + msz, :])
                knW2t.append(t)
            keysT = []   # 4 x [128h, 128k] fp32
            for hc in range(HC):
                pk = psetup.tile([128, 128], DT.float32, tag="pkeys")
                for mc, msz in enumerate(mc_sizes):
                    nc.tensor.matmul(pk[:],
                                     knW2t[mc][:, hc * 128:(hc + 1) * 128],
                                     kmT[mc][:], start=(mc == 0), stop=(mc == 2))
                kt = wpool.tile([128, MAXN], DT.float32, tag=f"keysT{hc}")
                if kn_b2_nonzero:
                    kb = ssb.tile([128, 1], DT.float32, tag="knb2c")
                    nc.sync.dma_start(kb[:],
                                      knb2_d[hc * 128:(hc + 1) * 128][:, None])
                    nc.scalar.activation(kt[:], pk[:], AF.Identity,
                                         bias=kb[:, 0:1])
                else:
                    nc.scalar.activation(kt[:], pk[:], AF.Identity)
                keysT.append(kt)

        # ------------------------------------------------------ main k loop
        ps1 = ctx.enter_context(tc.tile_pool(name="ps1", bufs=5, space="PSUM"))
        ps2 = ctx.enter_context(tc.tile_pool(name="ps2", bufs=3, space="PSUM"))
        zkp = ctx.enter_context(tc.tile_pool(name="zk", bufs=3))
        actp = ctx.enter_context(tc.tile_pool(name="act", bufs=2))
        outp = ctx.enter_context(tc.tile_pool(name="outsb", bufs=3))

        def _main_body():
          for k0 in range(0, K, 2):
            nk = min(2, K - k0)
            W = nk * BS
            # zk[hc] = zTh[hc] * keysT[hc][:, k]  (GpSimd, fp16, fp32 scalar)
            zk = [zkp.tile([128, W], DT.float16, tag=f"zk{hc}", name=f"zk{hc}")
                  for hc in range(HC)]
            for kk in range(nk):
                for hc in range(HC):
                    nc.vector.tensor_scalar(
                        zk[hc][:, kk * BS:(kk + 1) * BS], zTh[hc][:],
                        keysT[hc][:, k0 + kk:k0 + kk + 1], None, ALU.mult)
            a16 = actp.tile([128, DC * W], DT.float16, tag="a16")
            frags = []
            p1s = []
            for dc in range(DC):
                p1 = ps1.tile([128, W], DT.float32, tag="p1", name="p1")
                for hc in range(HC):
                    nc.tensor.matmul(p1[:], W1h[hc][:, dc * 128:(dc + 1) * 128],
                                     zk[hc][:], start=(hc == 0),
                                     stop=(hc == HC - 1))
                if de_b1_nonzero:
                    nc.vector.tensor_scalar(p1[:], p1[:], deb1c[dc][:, 0:1],
                                            None, ALU.add)
                frags.append((p1[:], a16[:, dc * W:(dc + 1) * W], W))
            _emit_mish(nc, mish_pools, frags, 128, DC * 512)
            out_sb = [outp.tile([128, W], DT.float32, tag=f"o{bt}", name=f"o{bt}")
                      for bt in range(2)]
            for kk in range(nk):
                for bt in range(2):
                    p2 = ps2.tile([128, DIM], DT.float32, tag="p2")
                    for dc in range(DC):
                        last = (dc == DC - 1) and not de_b2_nonzero
                        nc.tensor.matmul(
                            p2[:],
                            a16[:, dc * W + kk * BS + bt * 128:
                                dc * W + kk * BS + (bt + 1) * 128],
                            W2h[dc][:], start=(dc == 0), stop=last)
                    if de_b2_nonzero:
                        nc.tensor.matmul(p2[:], ones_row[:], deb2row[:],
                                         start=False, stop=True)
                    mcol = maskf[bt][:, k0 + kk:k0 + kk + 1]
                    dst = out_sb[bt][:, kk * DIM:(kk + 1) * DIM]
                    nc.scalar.activation(dst, p2[:], AF.Identity, scale=mcol)
            for bt in range(2):
                nc.sync.dma_start(
                    x_d[bt * 128:(bt + 1) * 128, k0:k0 + nk, :],
                    out_sb[bt][:, :W])

        if repeat > 1:
            with tc.For_i(0, repeat, 1):
                _main_body()
        else:
            _main_body()

    nc.compile()
    return nc


_NEFF_CACHE = {}


def kernel(**inputs):
    global LAST_RUN
    inp = {k: np.ascontiguousarray(np.asarray(v)) for k, v in inputs.items()}
    z = np.ascontiguousarray(inp["z"].astype(np.float32, copy=False))

    # --- host: size head -> n, mask, ragged bound K
    n = _host_size_pred(z, inp["sp_W1"], inp["sp_b1"], inp["sp_g"],
                        inp["sp_be"], inp["sp_W2"], inp["sp_b2"])
    mask = np.arange(MAXN)[None, :] < n[:, None]
    maskf = np.ascontiguousarray(mask.astype(np.float32))
    K = int(n.max())
    x = np.zeros((B, MAXN, DIM), np.float32)
    if K == 0:
        return x, n, mask

    # fold kn_b1 into kn_W1; if LN affine is non-trivial, fold the whole
    # key_net hidden layer on host (device then skips LN+mish).
    kn_w1eff = (inp["kn_W1"] + inp["kn_b1"]).astype(np.float32)
    kn_skip = False
    if not (np.all(inp["kn_g"] == 1.0) and np.all(inp["kn_be"] == 0.0)):
        kn_w1eff = _np_mish(_np_ln(kn_w1eff, inp["kn_g"], inp["kn_be"]))
        kn_skip = True

    de_b1_nonzero = bool(np.any(inp["de_b1"] != 0))
    de_b2_nonzero = bool(np.any(inp["de_b2"] != 0))
    kn_b2_nonzero = bool(np.any(inp["kn_b2"] != 0))

    ck = (K, de_b1_nonzero, de_b2_nonzero, kn_b2_nonzero, kn_skip)
    if ck not in _NEFF_CACHE:
        _NEFF_CACHE[ck] = build_kernel(*ck)
    nc = _NEFF_CACHE[ck]

    f32 = lambda a: np.ascontiguousarray(a.astype(np.float32, copy=False))
    shared = {
        "kn_W1": f32(kn_w1eff),
        "kn_W2": f32(inp["kn_W2"]),
        "kn_b2": f32(inp["kn_b2"]),
        "de_W1": f32(inp["de_W1"]),
        "de_b1": f32(inp["de_b1"]),
        "de_W2": f32(inp["de_W2"]),
        "de_b2": f32(inp["de_b2"]),
    }
    in_maps = [{**shared, "z": z[c * BS:(c + 1) * BS],
                "maskf": maskf[c * BS:(c + 1) * BS]} for c in range(NCORES)]

    LAST_RUN = run_bass_kernel_spmd(nc, in_maps, list(range(NCORES)))
    for c in range(NCORES):
        x[c * BS:(c + 1) * BS] = np.asarray(LAST_RUN.results[c]["x"])
    return x, n, mask


# revision 16
# speedup vs baseline: 3.7734x; 1.2198x over previous
"""Trainium2 Bass kernel for nn_Decoder (ragged sequence decoder).

Reference math:
  n      = clip(round(size_pred(z)), 0, 128)            [B]  (tiny scalar head)
  keys   = mish(LN(kn_W1 + kn_b1)) @ kn_W2 + kn_b2      [128, 512]
  x      = mish((z[:,None,:] * keys[None]) @ de_W1 + de_b1) @ de_W2 + de_b2
  x     *= (arange(128) < n[:,None])[..., None]         zero padded slots

Strategy: pure data parallel over batch (8 cores x 256 rows). The size head
(0.3% of FLOPs) runs on host in fp32 to build the ragged schedule: only
positions k < max(n) are computed on device; the rest of the output stays
zero via the pre-zeroed output buffers. keys are computed redundantly on
each device. The big decoder runs with fp16 matmul inputs and fp32 PSUM
accumulation. Per position k: x[:,k,:] = mish((z * keys[k]) @ W1) @ W2,
i.e. scale zT columns-of-keys into the moving matmul operand.

mish(x) = x*tanh(softplus(x)) has no HW activation table; it is computed
exactly via g = sigmoid(-x):  tanh(softplus(x)) = (1-g^2)/(1+g^2), so
  mish(x) = (r - 1) * x   with  r = 2/(1 + g^2)
using one ScalarE pass (Sigmoid), GpSimd passes (g^2, 0.5+0.5*g^2), and DVE
passes (reciprocal, fused (r-1)*x via scalar_tensor_tensor).
"""

import numpy as np
from contextlib import ExitStack

import concourse.bass as bass
import concourse.tile as tile
from concourse import bacc, mybir
from concourse.bass_utils import run_bass_kernel_spmd
from concourse.masks import make_identity

AF = mybir.ActivationFunctionType
ALU = mybir.AluOpType
DT = mybir.dt

B, DIM, HID, MAXN = 2048, 256, 512, 128
MID_S = (HID + 1) // 2      # 256
MID_K = (MAXN + HID) // 2   # 320
MID_D = (HID + DIM) // 2    # 384
NCORES = 8
BS = B // NCORES            # 256 rows per core
HC = HID // 128             # 4 h-chunks
DC = MID_D // 128           # 3 d-chunks
LN_EPS = 1e-5

LAST_RUN = None  # BassKernelResults of the last device launch (for profiling)


# ----------------------------------------------------------------- host math
def _np_mish(x):
    return (x * np.tanh(np.log1p(np.exp(x)))).astype(np.float32)


def _np_ln(x, g, b):
    m = x.mean(-1, keepdims=True, dtype=np.float32)
    v = x.var(-1, keepdims=True, dtype=np.float32)
    return ((x - m) / np.sqrt(v + LN_EPS) * g + b).astype(np.float32)


def _host_size_pred(z, sp_W1, sp_b1, sp_g, sp_be, sp_W2, sp_b2):
    h = _np_mish(_np_ln((z @ sp_W1 + sp_b1).astype(np.float32), sp_g, sp_be))
    nl = (h @ sp_W2 + sp_b2).astype(np.float32)
    return np.clip(np.round(nl[:, 0]), 0, MAXN).astype(np.int32)


# ------------------------------------------------------------- device kernel
def _act_recip(nc, out, in_, scale, bias):
    """ScalarE table reciprocal: out = 1/(in*scale + bias).

    bass.activation() refuses Reciprocal wholesale (generic accuracy
    concerns); our input range is exactly [1, 2] where the 1016-bucket
    table is accurate to ~1e-5 rel (hardware-probed), so emit directly.
    """
    eng = nc.scalar
    ins = [eng.lower_ap(in_)]
    for arg in (bias, scale, 0.0):
        ins.append(mybir.ImmediateValue(dtype=mybir.dt.float32, value=arg))
    return eng.add_instruction(mybir.InstActivation(
        name=nc.get_next_instruction_name(), func=AF.Reciprocal,
        ins=ins, outs=[eng.lower_ap(out)]))


def _emit_mish(nc, pools, frags, parts, width):
    """mish over PSUM fp32 fragments [(x_psum_ap, out16_ap, w), ...].

    mish(x) = (r - 1) * x,  r = 1/(0.5*sigmoid(-x)^2 + 0.5).
    Engine split: Sigmoid on ScalarE (single table set for the whole loop
    -- mixing table sets costs a ~10us reload per switch), g^2 and the
    affine on GpSimd, reciprocal + fused (r-1)*x on DVE.
    """
    g = pools["mg"].tile([parts, width], DT.float32, tag="mg")
    q = pools["mq"].tile([parts, width], DT.float32, tag="mq")
    d = pools["md"].tile([parts, width], DT.float32, tag="md")
    r = pools["mr"].tile([parts, width], DT.float32, tag="mr")
    off = 0
    for x_psum, out16, w in frags:
        gs, qs = g[:, off:off + w], q[:, off:off + w]
        ds, rs = d[:, off:off + w], r[:, off:off + w]
        nc.scalar.activation(gs, x_psum, AF.Sigmoid, scale=-1.0)
        nc.gpsimd.tensor_tensor(qs, gs, gs, ALU.mult)
        nc.gpsimd.tensor_scalar(ds, qs, 0.5, 0.5, ALU.mult, ALU.add)
        nc.vector.reciprocal(rs, ds)
        nc.vector.scalar_tensor_tensor(out16, rs, 1.0, x_psum, ALU.subtract,
                                       ALU.mult)
        off += w


def build_kernel(K, de_b1_nonzero, de_b2_nonzero, kn_b2_nonzero, kn_skip,
                 repeat=1):
    """One SPMD bass program computing x[:, :K, :] for a 256-row shard.

    repeat > 1 wraps the main loop in a hardware For-loop re-running the
    same (idempotent) computation; used only for wall-clock benchmarking.
    """
    nc = bacc.Bacc("TRN2", target_bir_lowering=False, debug=False,
                   num_devices=NCORES)

    z_d = nc.dram_tensor("z", [BS, HID], DT.float32, kind="ExternalInput").ap()
    maskf_d = nc.dram_tensor("maskf", [BS, MAXN], DT.float32,
                             kind="ExternalInput").ap()
    knW1_d = nc.dram_tensor("kn_W1", [MAXN, MID_K], DT.float32,
                            kind="ExternalInput").ap()
    knW2_d = nc.dram_tensor("kn_W2", [MID_K, HID], DT.float32,
                            kind="ExternalInput").ap()
    knb2_d = nc.dram_tensor("kn_b2", [HID], DT.float32, kind="ExternalInput").ap()
    deW1_d = nc.dram_tensor("de_W1", [HID, MID_D], DT.float32,
                            kind="ExternalInput").ap()
    deb1_d = nc.dram_tensor("de_b1", [MID_D], DT.float32, kind="ExternalInput").ap()
    deW2_d = nc.dram_tensor("de_W2", [MID_D, DIM], DT.float32,
                            kind="ExternalInput").ap()
    deb2_d = nc.dram_tensor("de_b2", [DIM], DT.float32, kind="ExternalInput").ap()
    x_d = nc.dram_tensor("x", [BS, MAXN, DIM], DT.float32,
                         kind="ExternalOutput").ap()

    with tile.TileContext(nc) as tc, ExitStack() as ctx:
        wpool = ctx.enter_context(tc.tile_pool(name="weights", bufs=1))
        mish_pools = {nm: ctx.enter_context(tc.tile_pool(name=nm, bufs=3))
                      for nm in ("mg", "mq", "md", "mr")}

        ident = wpool.tile([128, 128], DT.float32, tag="ident")
        make_identity(nc, ident[:])

        with tc.tile_pool(name="psetup", bufs=2, space="PSUM") as psetup, \
             tc.tile_pool(name="setup_sb", bufs=2) as ssb:
            # --- persistent weights (fp16)
            W1h = []   # de_W1, 4 x [128h, 384d]
            for hc in range(HC):
                t32 = ssb.tile([128, MID_D], DT.float32, tag="w1stage")
                nc.sync.dma_start(t32[:], deW1_d[hc * 128:(hc + 1) * 128, :])
                t16 = wpool.tile([128, MID_D], DT.float16, tag=f"w1h{hc}")
                nc.vector.tensor_copy(t16[:], t32[:])
                W1h.append(t16)
            W2h = []   # de_W2, 3 x [128d, 256]
            for dc in range(DC):
                t32 = ssb.tile([128, DIM], DT.float32, tag="w2stage")
                nc.sync.dma_start(t32[:], deW2_d[dc * 128:(dc + 1) * 128, :])
                t16 = wpool.tile([128, DIM], DT.float16, tag=f"w2h{dc}")
                nc.vector.tensor_copy(t16[:], t32[:])
                W2h.append(t16)

            deb1c = []
            if de_b1_nonzero:
                for dc in range(DC):
                    t = wpool.tile([128, 1], DT.float32, tag=f"deb1{dc}")
                    nc.sync.dma_start(t[:], deb1_d[dc * 128:(dc + 1) * 128][:, None])
                    deb1c.append(t)
            deb2row = ones_row = None
            if de_b2_nonzero:
                t32 = ssb.tile([1, DIM], DT.float32, tag="deb2st")
                nc.sync.dma_start(t32[:], deb2_d[None, :])
                deb2row = wpool.tile([1, DIM], DT.float16, tag="deb2h")
                nc.vector.tensor_copy(deb2row[:], t32[:])
                ones_row = wpool.tile([1, 128], DT.float16, tag="ones")
                nc.vector.memset(ones_row[:], 1.0)

            maskf = []  # 2 x [128b, 128k] fp32
            for bt in range(2):
                t = wpool.tile([128, MAXN], DT.float32, tag=f"maskf{bt}")
                nc.sync.dma_start(t[:], maskf_d[bt * 128:(bt + 1) * 128, :])
                maskf.append(t)

            # --- transpose z -> zTh fp16 [128h, 256b] x4
            zTh = [wpool.tile([128, BS], DT.float16, tag=f"zth{hc}", name=f"zth{hc}")
                   for hc in range(HC)]
            for bt in range(2):
                zrow = ssb.tile([128, HID], DT.float32, tag="zrow")
                nc.sync.dma_start(zrow[:], z_d[bt * 128:(bt + 1) * 128, :])
                for hc in range(HC):
                    pt = psetup.tile([128, 128], DT.float32, tag="ptr")
                    nc.tensor.transpose(pt[:], zrow[:, hc * 128:(hc + 1) * 128],
                                        ident[:])
                    nc.scalar.activation(zTh[hc][:, bt * 128:(bt + 1) * 128],
                                         pt[:], AF.Identity)

            # --- keys: km = mish(LN(kn_W1eff))   (kn_b1 folded on host)
            kw1 = ssb.tile([MAXN, MID_K], DT.float32, tag="kw1")
            nc.sync.dma_start(kw1[:], knW1_d)
            if kn_skip:
                km = kw1
            else:
                mean = ssb.tile([128, 1], DT.float32, tag="mean")
                nc.vector.tensor_reduce(mean[:], kw1[:], mybir.AxisListType.X,
                                        ALU.add)
                sq = ssb.tile([MAXN, MID_K], DT.float32, tag="sq")
                ssq = ssb.tile([128, 1], DT.float32, tag="ssq")
                nc.scalar.activation(sq[:], kw1[:], AF.Square, accum_out=ssq[:])
                nc.scalar.mul(mean[:], mean[:], 1.0 / MID_K)
                m2 = ssb.tile([128, 1], DT.float32, tag="m2")
                nc.scalar.activation(m2[:], mean[:], AF.Square)
                var = ssb.tile([128, 1], DT.float32, tag="var")
                nc.vector.scalar_tensor_tensor(var[:], ssq[:], 1.0 / MID_K,
                                               m2[:], ALU.mult, ALU.subtract)
                epsc = ssb.tile([128, 1], DT.float32, tag="epsc")
                nc.gpsimd.memset(epsc[:], LN_EPS)
                srt = ssb.tile([128, 1], DT.float32, tag="srt")
                nc.scalar.activation(srt[:], var[:], AF.Sqrt, bias=epsc[:, 0:1])
                rstd = ssb.tile([128, 1], DT.float32, tag="rstd")
                nc.vector.reciprocal(rstd[:], srt[:])
                nrm = ssb.tile([MAXN, MID_K], DT.float32, tag="nrm")
                nc.vector.tensor_scalar(nrm[:], kw1[:], mean[:, 0:1],
                                        rstd[:, 0:1], ALU.subtract, ALU.mult)
                # mish(nrm) in fp32 (same chain, fp32 output)
                g = mish_pools["mg"].tile([MAXN, MID_K], DT.float32, tag="mg")
                nc.scalar.activation(g[:], nrm[:], AF.Sigmoid, scale=-1.0)
                q = mish_pools["mq"].tile([MAXN, MID_K], DT.float32, tag="mq")
                nc.gpsimd.tensor_tensor(q[:], g[:], g[:], ALU.mult)
                d0 = mish_pools["md"].tile([MAXN, MID_K], DT.float32, tag="md")
                nc.gpsimd.tensor_scalar(d0[:], q[:], 0.5, 0.5, ALU.mult, ALU.add)
                r = mish_pools["mr"].tile([MAXN, MID_K], DT.float32, tag="mr")
                nc.vector.reciprocal(r[:], d0[:])
                km = ssb.tile([MAXN, MID_K], DT.float32, tag="km")
                nc.vector.scalar_tensor_tensor(km[:], r[:], 1.0, nrm[:],
                                               ALU.subtract, ALU.mult)

            # kmT chunks: 320m -> [128,128,64] partitions x [128k]
            mc_sizes = [128, 128, MID_K - 256]
            kmT = []
            for mc, msz in enumerate(mc_sizes):
                pt = psetup.tile([msz, 128], DT.float32, tag="pkmt")
                nc.tensor.transpose(pt[:], km[:, mc * 128:mc * 128 + msz],
                                    ident[:])
                t = ssb.tile([msz, 128], DT.float32, tag=f"kmt{mc}")
                nc.scalar.activation(t[:], pt[:], AF.Identity)
                kmT.append(t)

            knW2t = []
            for mc, msz in enumerate(mc_sizes):
                t = ssb.tile([msz, HID], DT.float32, tag=f"kw2{mc}")
                nc.sync.dma_start(t[:], knW2_d[mc * 128:mc * 128 + msz, :])
                knW2t.append(t)
            keysT = []   # 4 x [128h, 128k] fp32
            for hc in range(HC):
                pk = psetup.tile([128, 128], DT.float32, tag="pkeys")
                for mc, msz in enumerate(mc_sizes):
                    nc.tensor.matmul(pk[:],
                                     knW2t[mc][:, hc * 128:(hc + 1) * 128],
                                     kmT[mc][:], start=(mc == 0), stop=(mc == 2))
                kt = wpool.tile([128, MAXN], DT.float32, tag=f"keysT{hc}")
                if kn_b2_nonzero:
                    kb = ssb.tile([128, 1], DT.float32, tag="knb2c")
                    nc.sync.dma_start(kb[:],
                                      knb2_d[hc * 128:(hc + 1) * 128][:, None])
                    nc.scalar.activation(kt[:], pk[:], AF.Identity,
                                         bias=kb[:, 0:1])
                else:
                    nc.scalar.activation(kt[:], pk[:], AF.Identity)
                keysT.append(kt)

        # ------------------------------------------------------ main k loop
        ps1 = ctx.enter_context(tc.tile_pool(name="ps1", bufs=4, space="PSUM"))
        ps2 = ctx.enter_context(tc.tile_pool(name="ps2", bufs=4, space="PSUM"))
        zkp = ctx.enter_context(tc.tile_pool(name="zk", bufs=4))
        actp = ctx.enter_context(tc.tile_pool(name="act", bufs=3))
        outp = ctx.enter_context(tc.tile_pool(name="outsb", bufs=4))

        def _main_body():
          for k0 in range(0, K, 2):
            nk = min(2, K - k0)
            W = nk * BS
            # zk[hc] = zTh[hc] * keysT[hc][:, k]  (GpSimd, fp16, fp32 scalar)
            zk = [zkp.tile([128, W], DT.float16, tag=f"zk{hc}", name=f"zk{hc}")
                  for hc in range(HC)]
            for kk in range(nk):
                for hc in range(HC):
                    nc.vector.tensor_scalar(
                        zk[hc][:, kk * BS:(kk + 1) * BS], zTh[hc][:],
                        keysT[hc][:, k0 + kk:k0 + kk + 1], None, ALU.mult)
            a16 = actp.tile([128, DC * W], DT.float16, tag="a16")
            frags = []
            p1s = []
            for dc in range(DC):
                p1 = ps1.tile([128, W], DT.float32, tag="p1", name="p1")
                for hc in range(HC):
                    nc.tensor.matmul(p1[:], W1h[hc][:, dc * 128:(dc + 1) * 128],
                                     zk[hc][:], start=(hc == 0),
                                     stop=(hc == HC - 1))
                if de_b1_nonzero:
                    nc.vector.tensor_scalar(p1[:], p1[:], deb1c[dc][:, 0:1],
                                            None, ALU.add)
                frags.append((p1[:], a16[:, dc * W:(dc + 1) * W], W))
            _emit_mish(nc, mish_pools, frags, 128, DC * 512)
            out_sb = [outp.tile([128, W], DT.float32, tag=f"o{bt}", name=f"o{bt}")
                      for bt in range(2)]
            for kk in range(nk):
                for bt in range(2):
                    p2 = ps2.tile([128, DIM], DT.float32, tag="p2")
                    for dc in range(DC):
                        last = (dc == DC - 1) and not de_b2_nonzero
                        nc.tensor.matmul(
                            p2[:],
                            a16[:, dc * W + kk * BS + bt * 128:
                                dc * W + kk * BS + (bt + 1) * 128],
                            W2h[dc][:], start=(dc == 0), stop=last)
                    if de_b2_nonzero:
                        nc.tensor.matmul(p2[:], ones_row[:], deb2row[:],
                                         start=False, stop=True)
                    mcol = maskf[bt][:, k0 + kk:k0 + kk + 1]
                    dst = out_sb[bt][:, kk * DIM:(kk + 1) * DIM]
                    nc.scalar.activation(dst, p2[:], AF.Identity, scale=mcol)
            for bt in range(2):
                nc.sync.dma_start(
                    x_d[bt * 128:(bt + 1) * 128, k0:k0 + nk, :],
                    out_sb[bt][:, :W])

        if repeat > 1:
            with tc.For_i(0, repeat, 1):
                _main_body()
        else:
            _main_body()

    nc.compile()
    return nc


_NEFF_CACHE = {}


def kernel(**inputs):
    global LAST_RUN
    inp = {k: np.ascontiguousarray(np.asarray(v)) for k, v in inputs.items()}
    z = np.ascontiguousarray(inp["z"].astype(np.float32, copy=False))

    # --- host: size head -> n, mask, ragged bound K
    n = _host_size_pred(z, inp["sp_W1"], inp["sp_b1"], inp["sp_g"],
                        inp["sp_be"], inp["sp_W2"], inp["sp_b2"])
    mask = np.arange(MAXN)[None, :] < n[:, None]
    maskf = np.ascontiguousarray(mask.astype(np.float32))
    K = int(n.max())
    x = np.zeros((B, MAXN, DIM), np.float32)
    if K == 0:
        return x, n, mask

    # fold kn_b1 into kn_W1; if LN affine is non-trivial, fold the whole
    # key_net hidden layer on host (device then skips LN+mish).
    kn_w1eff = (inp["kn_W1"] + inp["kn_b1"]).astype(np.float32)
    kn_skip = False
    if not (np.all(inp["kn_g"] == 1.0) and np.all(inp["kn_be"] == 0.0)):
        kn_w1eff = _np_mish(_np_ln(kn_w1eff, inp["kn_g"], inp["kn_be"]))
        kn_skip = True

    de_b1_nonzero = bool(np.any(inp["de_b1"] != 0))
    de_b2_nonzero = bool(np.any(inp["de_b2"] != 0))
    kn_b2_nonzero = bool(np.any(inp["kn_b2"] != 0))

    ck = (K, de_b1_nonzero, de_b2_nonzero, kn_b2_nonzero, kn_skip)
    if ck not in _NEFF_CACHE:
        _NEFF_CACHE[ck] = build_kernel(*ck)
    nc = _NEFF_CACHE[ck]

    f32 = lambda a: np.ascontiguousarray(a.astype(np.float32, copy=False))
    shared = {
        "kn_W1": f32(kn_w1eff),
        "kn_W2": f32(inp["kn_W2"]),
        "kn_b2": f32(inp["kn_b2"]),
        "de_W1": f32(inp["de_W1"]),
        "de_b1": f32(inp["de_b1"]),
        "de_W2": f32(inp["de_W2"]),
        "de_b2": f32(inp["de_b2"]),
    }
    in_maps = [{**shared, "z": z[c * BS:(c + 1) * BS],
                "maskf": maskf[c * BS:(c + 1) * BS]} for c in range(NCORES)]

    LAST_RUN = run_bass_kernel_spmd(nc, in_maps, list(range(NCORES)))
    for c in range(NCORES):
        x[c * BS:(c + 1) * BS] = np.asarray(LAST_RUN.results[c]["x"])
    return x, n, mask
